# revision 1
# baseline (speedup 1.0000x reference)
"""GraphTransformerLayer on 8 Trainium2 NeuronCores (Bass/Tile).

Sharding: 8-way along the query-node axis. Each core owns NQ=512 query rows,
computes the full K/V projections (replicated), its slice of the masked
attention, and its slice of the FFN. No collectives needed; the host
concatenates the 8 output slices.

Attention dataflow (per core, per head h):
  sT[m, n]  = sum_d k[m, d] q[n, d]        (PE; K=dk=64, out [128m, 512n])
  aT        = exp(0.125 * sT)              (ACT, PSUM->SBUF bf16)
  aT       *= maskT[m, n]                  (DVE, bf16)
  ctxT_ext  = [v_h | 1].T @ aT             (PE; out [65, 512n], row 64 = rowsum)
  ctxT_h    = ctxT_ext[0:64] * (1/rowsum)  (DVE; recip bcast via GPSIMD)
ctxT is directly the lhsT for the Wo matmul. h1/h2 residual+LN in f32.
"""

import sys

if "/opt/trn_rl_repo" not in sys.path:
    sys.path.insert(0, "/opt/trn_rl_repo")

import numpy as np
import ml_dtypes

import concourse.bacc as bacc
import concourse.tile as tile
import concourse.mybir as mybir
from concourse.bass_utils import run_bass_kernel_spmd

BF16 = ml_dtypes.bfloat16
F32 = mybir.dt.float32
BF = mybir.dt.bfloat16

N = 4096
D = 512
H = 8
DK = 64
DFF = 2048
NCORES = 8
NQ = N // NCORES  # 512 query rows per core
P = 128
EPS = 1e-5

ALU = mybir.AluOpType
AF = mybir.ActivationFunctionType

# set by test.py to capture a profile
TRACE = False
TRACE_DIR = None
LAST_EXEC_NS = None

# debug: truncate the program after a phase (1=proj, 2=attention, 3=ln1, 4=full)
STOP_AT = 4

_CACHED = None


def _build():
    nc = bacc.Bacc("TRN2", target_bir_lowering=False, debug=False,
                   num_devices=NCORES)

    # ---- DRAM I/O ----
    hT = nc.dram_tensor("hT", [D, N], BF, kind="ExternalInput").ap()
    hqT = nc.dram_tensor("hqT", [D, NQ], BF, kind="ExternalInput").ap()
    hq = nc.dram_tensor("hq", [NQ, D], F32, kind="ExternalInput").ap()
    maskT = nc.dram_tensor("maskT", [N, NQ], BF, kind="ExternalInput").ap()
    wqT = nc.dram_tensor("wqT", [D, D], BF, kind="ExternalInput").ap()
    wkT = nc.dram_tensor("wkT", [D, D], BF, kind="ExternalInput").ap()
    wvT = nc.dram_tensor("wvT", [D, D], BF, kind="ExternalInput").ap()
    woT = nc.dram_tensor("woT", [D, D], BF, kind="ExternalInput").ap()
    w1T = nc.dram_tensor("w1T", [D, DFF], BF, kind="ExternalInput").ap()
    w2T = nc.dram_tensor("w2T", [DFF, D], BF, kind="ExternalInput").ap()
    bq = nc.dram_tensor("bq", [D], F32, kind="ExternalInput").ap()
    bk = nc.dram_tensor("bk", [D], F32, kind="ExternalInput").ap()
    b1 = nc.dram_tensor("b1", [DFF], F32, kind="ExternalInput").ap()
    bv2 = nc.dram_tensor("bv2", [1, D], BF, kind="ExternalInput").ap()
    b22 = nc.dram_tensor("b22", [1, D], BF, kind="ExternalInput").ap()
    g1b = nc.dram_tensor("g1b", [P, D], F32, kind="ExternalInput").ap()
    b1b = nc.dram_tensor("b1b", [P, D], F32, kind="ExternalInput").ap()
    g2b = nc.dram_tensor("g2b", [P, D], F32, kind="ExternalInput").ap()
    b2b = nc.dram_tensor("b2b", [P, D], F32, kind="ExternalInput").ap()
    ident = nc.dram_tensor("ident", [P, P], F32, kind="ExternalInput").ap()
    out = nc.dram_tensor("out", [NQ, D], F32, kind="ExternalOutput").ap()

    with tile.TileContext(nc) as tc:
        _emit(nc, tc, locals())
    nc.compile()
    return nc


def _emit(nc, tc, t):
    hT, hqT, hq, maskT = t["hT"], t["hqT"], t["hq"], t["maskT"]
    wqT, wkT, wvT, woT = t["wqT"], t["wkT"], t["wvT"], t["woT"]
    w1T, w2T = t["w1T"], t["w2T"]
    bq, bk, b1, bv2, b22 = t["bq"], t["bk"], t["b1"], t["bv2"], t["b22"]
    g1b, b1b, g2b, b2b = t["g1b"], t["b1b"], t["g2b"], t["b2b"]
    ident, out = t["ident"], t["out"]

    from contextlib import ExitStack

    es = ExitStack()
    with es:
        cpool = es.enter_context(tc.tile_pool(name="const", bufs=1))
        h1pool = es.enter_context(tc.tile_pool(name="h1p", bufs=1))
        qkv_es = ExitStack()
        mpool = qkv_es.enter_context(tc.tile_pool(name="maskp", bufs=1))
        qkvpool = qkv_es.enter_context(tc.tile_pool(name="qkvp", bufs=1))

        # ---- constants ----
        bq_sb = cpool.tile([P, 4], F32, tag="bq")
        nc.sync.dma_start(bq_sb[:], bq.rearrange("(t p) -> p t", p=P))
        bk_sb = cpool.tile([P, 4], F32, tag="bk")
        nc.sync.dma_start(bk_sb[:], bk.rearrange("(t p) -> p t", p=P))
        b1_sb = cpool.tile([P, 16], F32, tag="b1")
        nc.sync.dma_start(b1_sb[:], b1.rearrange("(t p) -> p t", p=P))
        bv_sb = cpool.tile([1, D], BF, tag="bv")
        nc.sync.dma_start(bv_sb[:], bv2[:])
        b2_sb = cpool.tile([1, D], BF, tag="b2")
        nc.sync.dma_start(b2_sb[:], b22[:])
        ident_sb = cpool.tile([P, P], F32, tag="id")
        nc.sync.dma_start(ident_sb[:], ident[:])
        identb_sb = cpool.tile([P, P], BF, tag="idb")
        nc.vector.tensor_copy(identb_sb[:], ident_sb[:])
        ones_sb = cpool.tile([1, P], BF, tag="ones")
        nc.vector.memset(ones_sb[:], 1.0)
        eps_sb = cpool.tile([P, 1], F32, tag="eps")
        nc.vector.memset(eps_sb[:], EPS)
        ln_sb = {}
        for nm, src in (("g1", g1b), ("b1l", b1b), ("g2", g2b), ("b2l", b2b)):
            tl = cpool.tile([P, D], F32, tag=nm, name=f"ln_{nm}")
            nc.sync.dma_start(tl[:], src[:])
            ln_sb[nm] = tl
        hq_sb = []
        for qt in range(4):
            tl = cpool.tile([P, D], F32, tag=f"hq{qt}", name=f"hq{qt}")
            nc.sync.dma_start(tl[:], hq[qt * P:(qt + 1) * P, :])
            hq_sb.append(tl)
        woT_sb = []
        for s in range(4):
            tl = cpool.tile([P, D], BF, tag=f"wo{s}", name=f"wo{s}")
            nc.sync.dma_start(tl[:], woT[s * P:(s + 1) * P, :])
            woT_sb.append(tl)

        # ---- persistent qkv outputs ----
        kT_sb = [qkvpool.tile([P, N], BF, tag=f"kt{i}", name=f"kT{i}")
                 for i in range(4)]
        qT_sb = [qkvpool.tile([P, NQ], BF, tag=f"qt{i}", name=f"qT{i}")
                 for i in range(4)]
        # v_ext[mt]: [128, 8*65]; per head h cols h*65..h*65+64, col 64 = ones
        v_sb = [qkvpool.tile([P, H * (DK + 1)], BF, tag=f"v{i}", name=f"v{i}")
                for i in range(32)]

        # ================= projections =================
        with tc.tile_pool(name="projp", bufs=1) as ppool, \
             tc.tile_pool(name="psproj", bufs=4, space="PSUM") as psp:
            wq_sb, wk_sb, wv_sb, hqT_sb = [], [], [], []
            for s in range(4):
                tl = ppool.tile([P, D], BF, tag=f"wq{s}", name=f"wq{s}")
                nc.sync.dma_start(tl[:], wqT[s * P:(s + 1) * P, :])
                wq_sb.append(tl)
                tl = ppool.tile([P, D], BF, tag=f"wk{s}", name=f"wk{s}")
                nc.sync.dma_start(tl[:], wkT[s * P:(s + 1) * P, :])
                wk_sb.append(tl)
                tl = ppool.tile([P, D], BF, tag=f"wv{s}", name=f"wv{s}")
                nc.sync.dma_start(tl[:], wvT[s * P:(s + 1) * P, :])
                wv_sb.append(tl)
                tl = ppool.tile([P, NQ], BF, tag=f"hqT{s}", name=f"hqT{s}")
                nc.sync.dma_start(tl[:], hqT[s * P:(s + 1) * P, :])
                hqT_sb.append(tl)

            # mask DMA issued after the projection inputs so the first
            # matmuls are not stuck behind a 4MB transfer
            mask_sb = mpool.tile([P, 32 * NQ], BF, tag="mask")
            nc.gpsimd.dma_start(
                mask_sb.rearrange("p (mt j) -> p mt j", j=NQ),
                maskT.rearrange("(mt p) j -> p mt j", p=P),
            )

            # qT[t] = (Wq @ hqT)[t-rows] + bq
            for tt in range(4):
                ps = psp.tile([P, NQ], F32, tag="pp", name="ps_q")
                for s in range(4):
                    nc.tensor.matmul(ps[:], wq_sb[s][:, tt * P:(tt + 1) * P],
                                     hqT_sb[s][:], start=(s == 0), stop=(s == 3))
                nc.scalar.activation(qT_sb[tt][:], ps[:], AF.Identity,
                                     bias=bq_sb[:, tt:tt + 1])

            # kT and v, streaming hT in two halves of 2048 columns
            for half in range(2):
                c0 = half * (N // 2)
                hT_sb = []
                for s in range(4):
                    tl = ppool.tile([P, N // 2], BF, tag=f"ht{s}", name=f"ht{s}")
                    nc.sync.dma_start(tl[:], hT[s * P:(s + 1) * P,
                                                c0:c0 + N // 2])
                    hT_sb.append(tl)
                for tt in range(4):
                    for c in range(4):
                        ps = psp.tile([P, 512], F32, tag="pp", name="ps_k")
                        for s in range(4):
                            nc.tensor.matmul(
                                ps[:], wk_sb[s][:, tt * P:(tt + 1) * P],
                                hT_sb[s][:, c * 512:(c + 1) * 512],
                                start=(s == 0), stop=(s == 3))
                        nc.vector.tensor_scalar_add(
                            kT_sb[tt][:, c0 + c * 512:c0 + (c + 1) * 512],
                            ps[:], bk_sb[:, tt:tt + 1])
                for mtl in range(16):
                    mt = half * 16 + mtl
                    ps = psp.tile([P, D], F32, tag="pp", name="ps_v")
                    for s in range(4):
                        nc.tensor.matmul(ps[:],
                                         hT_sb[s][:, mtl * P:(mtl + 1) * P],
                                         wv_sb[s][:], start=(s == 0),
                                         stop=False)
                    nc.tensor.matmul(ps[:], ones_sb[:], bv_sb[:],
                                     start=False, stop=True)
                    vv = v_sb[mt].rearrange("p (h c) -> p h c", c=DK + 1)
                    nc.scalar.copy(vv[:, :, 0:DK],
                                   ps.rearrange("p (h c) -> p h c", c=DK))
                    nc.vector.memset(vv[:, :, DK:DK + 1], 1.0)

        if STOP_AT == 1:
            # anchor projections: out[qt] = f32(kT[qt][:, :512] + qT) + v
            for qt in range(4):
                cv = h1pool.tile([P, D], F32, tag=f"x{qt}", bufs=2, name="cv")
                nc.vector.tensor_add(cv[:], kT_sb[qt][:, 0:D], qT_sb[qt][:])
                nc.vector.tensor_add(cv[:], cv[:], v_sb[qt * 8][:, 0:D])
                nc.sync.dma_start(out[qt * P:(qt + 1) * P, :], cv[:])
            qkv_es.close()
            return

        # ================= attention =================
        with tc.tile_pool(name="attp", bufs=1) as apool:
            ctxT_sb = [apool.tile([P, NQ], BF, tag=f"cx{i}", name=f"ctxT{i}")
                       for i in range(4)]
            with tc.tile_pool(name="psatt", bufs=1, space="PSUM") as psa:
                for hp in range(4):
                    h0, h1_ = 2 * hp, 2 * hp + 1
                    ctx_ps = [psa.tile([P, NQ], F32, tag="pc", bufs=2,
                                       name="ctx_ps") for _ in range(2)]
                    for g in range(16):
                        sp = [psa.tile([P, 1024], F32, tag="ps", bufs=3,
                                       name="sc_ps") for _ in range(2)]
                        at = [apool.tile([P, 1024], BF, tag="at", bufs=6,
                                         name="at") for _ in range(2)]
                        for i, po in ((0, 0), (1, DK)):
                            for j in range(2):
                                mt = 2 * g + j
                                nc.tensor.matmul(
                                    sp[i][:, j * NQ:(j + 1) * NQ],
                                    kT_sb[hp][po:po + DK, mt * P:(mt + 1) * P],
                                    qT_sb[hp][po:po + DK, :],
                                    start=True, stop=True)
                        for i in range(2):
                            nc.scalar.activation(at[i][:], sp[i][:], AF.Exp,
                                                 scale=0.125)
                            nc.vector.tensor_mul(
                                at[i][:], at[i][:],
                                mask_sb[:, g * 1024:(g + 1) * 1024])
                        for i, h in ((0, h0), (1, h1_)):
                            for j in range(2):
                                mt = 2 * g + j
                                nc.tensor.matmul(
                                    ctx_ps[i][0:DK + 1, :],
                                    v_sb[mt][:, h * 65:h * 65 + 65],
                                    at[i][:, j * NQ:(j + 1) * NQ],
                                    start=(mt == 0), stop=(mt == 31))
                    # normalize: ctxT_h = ctx[0:64] * (1/rowsum)
                    for i, po in ((0, 0), (1, DK)):
                        rec = apool.tile([1, NQ], F32, tag="rec", bufs=2,
                                         name="rec")
                        nc.vector.reciprocal(rec[:], ctx_ps[i][DK:DK + 1, :])
                        bc = apool.tile([P, NQ], F32, tag="bc", bufs=2,
                                        name="bc")
                        nc.gpsimd.partition_broadcast(bc[:], rec[:])
                        nc.vector.tensor_mul(ctxT_sb[hp][po:po + DK, :],
                                             ctx_ps[i][0:DK, :],
                                             bc[0:DK, :])

            if STOP_AT == 2:
                for qt in range(4):
                    cv = h1pool.tile([P, D], F32, tag=f"x{qt}", bufs=2,
                                     name="cv")
                    nc.vector.tensor_copy(cv[:], ctxT_sb[qt][:])
                    nc.sync.dma_start(out[qt * P:(qt + 1) * P, :], cv[:])

            # ---- Wo + residual + LN1 + transpose ----
            h1_sb = [h1pool.tile([P, D], F32, tag=f"h1_{i}", name=f"h1_{i}")
                     for i in range(4)]
            h1T_sb = [h1pool.tile([P, NQ], BF, tag=f"h1T{i}", name=f"h1T{i}")
                      for i in range(4)]
            with tc.tile_pool(name="pspost", bufs=2, space="PSUM") as psw:
                for qt in range(4 if STOP_AT > 2 else 0):
                    ps = psw.tile([P, D], F32, tag="po", name="wo_ps")
                    for s in range(4):
                        nc.tensor.matmul(ps[:],
                                         ctxT_sb[s][:, qt * P:(qt + 1) * P],
                                         woT_sb[s][:], start=(s == 0),
                                         stop=(s == 3))
                    if STOP_AT == 30:
                        nc.vector.tensor_add(h1_sb[qt][:], ps[:], hq_sb[qt][:])
                        continue
                    h1 = _layer_norm(nc, h1pool, qt, ps, hq_sb[qt],
                                     ln_sb["g1"], ln_sb["b1l"], h1_sb[qt],
                                     eps_sb, stop_at=STOP_AT)
                    if STOP_AT in (31, 32, 311, 312, 313, 3110, 3111):
                        continue
                    h1b = h1pool.tile([P, D], BF, tag="h1b", bufs=2,
                                      name="h1b")
                    nc.vector.tensor_copy(h1b[:], h1[:])
                    for i in range(4):
                        tp = psw.tile([P, P], BF, tag="tp", name="tp")
                        nc.tensor.transpose(tp[:], h1b[:, i * P:(i + 1) * P],
                                            identb_sb[:])
                        nc.vector.tensor_copy(
                            h1T_sb[i][:, qt * P:(qt + 1) * P], tp[:])

        qkv_es.close()

        if STOP_AT == 2:
            return
        if STOP_AT in (3, 30, 31, 32, 311, 312, 313, 3110, 3111):
            for qt in range(4):
                nc.sync.dma_start(out[qt * P:(qt + 1) * P, :], h1_sb[qt][:])
            return

        # ================= FFN =================
        with tc.tile_pool(name="ffnp", bufs=1) as fpool, \
             tc.tile_pool(name="psffn", bufs=4, space="PSUM") as psf:
            w1_sb = []
            for s in range(4):
                tl = fpool.tile([P, DFF], BF, tag=f"w1_{s}", name=f"w1_{s}")
                nc.sync.dma_start(tl[:], w1T[s * P:(s + 1) * P, :])
                w1_sb.append(tl)
            w2_sb = []
            for ft in range(16):
                tl = fpool.tile([P, D], BF, tag=f"w2_{ft}", name=f"w2_{ft}")
                nc.sync.dma_start(tl[:], w2T[ft * P:(ft + 1) * P, :])
                w2_sb.append(tl)
            fT_sb = [fpool.tile([P, NQ], BF, tag=f"fT{i}", name=f"fT{i}")
                     for i in range(16)]
            for ft in range(16):
                ps = psf.tile([P, NQ], F32, tag="pf", name="f_ps")
                for s in range(4):
                    nc.tensor.matmul(ps[:], w1_sb[s][:, ft * P:(ft + 1) * P],
                                     h1T_sb[s][:], start=(s == 0),
                                     stop=(s == 3))
                nc.scalar.activation(fT_sb[ft][:], ps[:], AF.Relu,
                                     bias=b1_sb[:, ft:ft + 1])
            for qt in range(4):
                ps = psf.tile([P, D], F32, tag="pf", name="ff_ps")
                for ft in range(16):
                    nc.tensor.matmul(ps[:], fT_sb[ft][:, qt * P:(qt + 1) * P],
                                     w2_sb[ft][:], start=(ft == 0), stop=False)
                nc.tensor.matmul(ps[:], ones_sb[:], b2_sb[:],
                                 start=False, stop=True)
                h2 = _layer_norm(nc, h1pool, qt + 4, ps, h1_sb[qt],
                                 ln_sb["g2"], ln_sb["b2l"], None, eps_sb)
                nc.sync.dma_start(out[qt * P:(qt + 1) * P, :], h2[:])


def _layer_norm(nc, pool, uid, z_ps, res_sb, g_sb, b_sb, out_tile, eps_sb,
                stop_at=4):
    """out = LN(z_ps + res_sb) * g + b, f32. Returns the output tile."""
    x = pool.tile([P, D], F32, tag=f"x{uid % 4}", bufs=2, name=f"x{uid}")
    s1 = pool.tile([P, 1], F32, tag="s1", bufs=4, name="s1")
    nc.vector.scalar_tensor_tensor(x[:], z_ps[:], 0.0, res_sb[:],
                                   op0=ALU.add, op1=ALU.add, accum_out=s1[:])
    if stop_at == 3110:
        nc.vector.tensor_scalar_add(out_tile[:], x[:], s1[:])
        return out_tile
    xsq = pool.tile([P, D], F32, tag="xsq", bufs=2, name="xsq")
    s2 = pool.tile([P, 1], F32, tag="s2", bufs=4, name="s2")
    nc.vector.tensor_mul(xsq[:], x[:], x[:])
    nc.vector.reduce_sum(s2[:], xsq[:], axis=mybir.AxisListType.X)
    if stop_at == 3111:
        nc.vector.tensor_scalar_add(out_tile[:], xsq[:], s2[:])
        return out_tile
    if stop_at == 311:
        nc.vector.tensor_scalar(out_tile[:], xsq[:], s2[:], s1[:],
                                op0=ALU.add, op1=ALU.add)
        return out_tile
    nm = pool.tile([P, 1], F32, tag="nm", bufs=4, name="nm")
    nc.vector.tensor_scalar_mul(nm[:], s1[:], -1.0 / D)
    m2 = pool.tile([P, 1], F32, tag="m2", bufs=4, name="m2")
    nc.vector.tensor_mul(m2[:], nm[:], nm[:])
    var = pool.tile([P, 1], F32, tag="var", bufs=4, name="var")
    nc.vector.scalar_tensor_tensor(var[:], s2[:], 1.0 / D, m2[:],
                                   op0=ALU.mult, op1=ALU.subtract)
    if stop_at == 312:
        nc.vector.tensor_scalar(out_tile[:], x[:], var[:], None, op0=ALU.add)
        return out_tile
    # rstd = rsqrt(var + eps), pure DVE: bit-trick seed + 3 Newton steps
    I32 = mybir.dt.int32
    ve = pool.tile([P, 1], F32, tag="ve", bufs=4, name="ve")
    nc.vector.tensor_scalar_add(ve[:], var[:], eps_sb[:])
    rstd = pool.tile([P, 1], F32, tag="rstd", bufs=4, name="rstd")
    nc.vector.tensor_single_scalar(rstd[:].bitcast(I32), ve[:].bitcast(I32),
                                   1, op=ALU.arith_shift_right)
    nc.vector.tensor_single_scalar(rstd[:].bitcast(I32), rstd[:].bitcast(I32),
                                   0x5F3759DF, op=ALU.subtract)
    nc.vector.tensor_single_scalar(rstd[:].bitcast(I32), rstd[:].bitcast(I32),
                                   -1, op=ALU.mult)
    tq = pool.tile([P, 1], F32, tag="tq", bufs=4, name="tq")
    for _ in range(3):
        nc.vector.tensor_mul(tq[:], rstd[:], rstd[:])
        nc.vector.tensor_mul(tq[:], tq[:], ve[:])
        nc.vector.tensor_scalar_mul(tq[:], tq[:], -0.5)
        nc.vector.tensor_scalar_add(tq[:], tq[:], 1.5)
        nc.vector.tensor_mul(rstd[:], rstd[:], tq[:])
    if stop_at == 313:
        nc.vector.tensor_scalar(out_tile[:], x[:], rstd[:], None, op0=ALU.add)
        return out_tile
    # xn = (x - mean) * rstd, in place
    nc.vector.tensor_scalar_add(x[:], x[:], nm[:])
    nc.vector.tensor_scalar_mul(x[:], x[:], rstd[:])
    if stop_at == 31:
        nc.vector.tensor_copy(out_tile[:], x[:])
        return out_tile
    if out_tile is None:
        out_tile = pool.tile([P, D], F32, tag=f"x{uid % 4}", bufs=2,
                             name=f"h2_{uid}")
    nc.vector.tensor_mul(out_tile[:], x[:], g_sb[:])
    nc.vector.tensor_add(out_tile[:], out_tile[:], b_sb[:])
    return out_tile


def _prep_inputs(inputs):
    h = np.asarray(inputs["h"], np.float32)
    adj = np.asarray(inputs["adj"])
    f32 = np.float32

    def bf(x):
        return np.ascontiguousarray(np.asarray(x, np.float32).astype(BF16))

    hT_full = bf(h.T)
    adjb = (adj != 0)
    np.fill_diagonal(adjb, True)
    adjb_bf = adjb.astype(BF16)

    wq, wk, wv, wo = (np.asarray(inputs[k], f32)
                      for k in ("Wq", "Wk", "Wv", "Wo"))
    w1, w2 = np.asarray(inputs["W1"], f32), np.asarray(inputs["W2"], f32)
    shared = {
        "hT": hT_full,
        "wqT": bf(wq.T), "wkT": bf(wk.T), "wvT": bf(wv.T), "woT": bf(wo.T),
        "w1T": bf(w1.T), "w2T": bf(w2.T),
        "bq": np.ascontiguousarray(np.asarray(inputs["bq"], f32)),
        "bk": np.ascontiguousarray(np.asarray(inputs["bk"], f32)),
        "b1": np.ascontiguousarray(np.asarray(inputs["b1"], f32)),
        "bv2": bf(np.asarray(inputs["bv"], f32)[None, :]),
        "b22": bf(np.asarray(inputs["b2"], f32)[None, :]),
        "g1b": np.ascontiguousarray(
            np.broadcast_to(np.asarray(inputs["ln1_g"], f32), (P, D))),
        "b1b": np.ascontiguousarray(
            np.broadcast_to(np.asarray(inputs["ln1_b"], f32), (P, D))),
        "g2b": np.ascontiguousarray(
            np.broadcast_to(np.asarray(inputs["ln2_g"], f32), (P, D))),
        "b2b": np.ascontiguousarray(
            np.broadcast_to(np.asarray(inputs["ln2_b"], f32), (P, D))),
        "ident": np.eye(P, dtype=f32),
    }
    bo = np.asarray(inputs["bo"], f32)
    in_maps = []
    for i in range(NCORES):
        r0 = i * NQ
        m = dict(shared)
        m["hqT"] = np.ascontiguousarray(hT_full[:, r0:r0 + NQ])
        m["hq"] = np.ascontiguousarray(h[r0:r0 + NQ, :] + bo)
        m["maskT"] = np.ascontiguousarray(adjb_bf[r0:r0 + NQ, :].T)
        in_maps.append(m)
    return in_maps


def kernel(**inputs) -> np.ndarray:
    global _CACHED, LAST_EXEC_NS
    if _CACHED is None:
        _CACHED = _build()
    nc = _CACHED
    in_maps = _prep_inputs(inputs)
    kw = {}
    if TRACE:
        kw = dict(trace=True, tmpdir=TRACE_DIR)
    res = run_bass_kernel_spmd(nc, in_maps, list(range(NCORES)), **kw)
    LAST_EXEC_NS = res.exec_time_ns
    return np.concatenate([res.results[i]["out"] for i in range(NCORES)],
                          axis=0)



# revision 10
# speedup vs baseline: 1.0385x; 1.0385x over previous
"""GraphTransformerLayer on 8 Trainium2 NeuronCores (Bass/Tile).

Sharding: 8-way along the query-node axis. Each core owns NQ=512 query rows,
computes full K/V projections (replicated), its slice of masked attention,
and its slice of the FFN. No collectives; the host concatenates the slices.

v2 vs baseline:
- Q/K/V and Wo matmuls run in fp8e4 DoubleRow perf mode (2 k-subtiles per
  instruction, 0.5 cycles/row): weights host-folded to [128, 4, .] and
  scaled x32 (x64 for the on-device ctx operand) to stay in fp8 range.
- bk dropped (softmax shift-invariance makes it exact), bv folded into a
  host-precomputed bo' = bo + bv @ Wo.T.
- Wo streams into an SBUF f32 accumulator per head-pair instead of running
  after all attention finishes.
- softmax normalization uses one reciprocal_approx_fast per head (the exact
  DVE reciprocal cost 3.3us per call).
- FFN1/FFN2 interleaved per ft-tile; LN2 + output DMA per query tile.
- DMAs issued in first-use order; w1/w2 batched into single transfers.

Attention dataflow per core, head h (unchanged):
  sT[m, n] = sum_d k[m,d] q[n,d]          (PE, K=64, out [128m, 512n])
  aT       = exp(0.125 * sT)              (ACT, PSUM->SBUF bf16)
  aT      *= maskT[m, n]                  (DVE, bf16)
  ctxT_ext = [v_h | 1].T @ aT             (PE, out [65, 512], row 64 = rowsum)
  ctxT2_h  = ctx[0:64] * (64/rowsum)      (DVE stt, fp8 out; recip via gpsimd
                                           partition_broadcast)
"""

import sys

if "/opt/trn_rl_repo" not in sys.path:
    sys.path.insert(0, "/opt/trn_rl_repo")

import numpy as np
import ml_dtypes

import concourse.bacc as bacc
import concourse.tile as tile
import concourse.mybir as mybir
from concourse.bass_utils import run_bass_kernel_spmd

BF16 = ml_dtypes.bfloat16
FP8 = ml_dtypes.float8_e4m3
F32 = mybir.dt.float32
BF = mybir.dt.bfloat16
F8 = mybir.dt.float8e4

N = 4096
D = 512
H = 8
DK = 64
DFF = 2048
NCORES = 8
NQ = N // NCORES
P = 128
EPS = 1e-5
WS = 32.0  # host weight pre-scale for fp8
CS = 64.0  # ctx pre-scale for fp8

ALU = mybir.AluOpType
AF = mybir.ActivationFunctionType
DR = mybir.MatmulPerfMode.DoubleRow

# set by test.py to capture a profile
TRACE = False
TRACE_DIR = None
LAST_EXEC_NS = None

# debug: truncate after a phase (1=proj, 2=attention ctx, 3=h1acc, 4=full)
STOP_AT = 4

_CACHED = None


def _build():
    nc = bacc.Bacc("TRN2", target_bir_lowering=False, debug=False,
                   num_devices=NCORES)

    # ---- DRAM I/O ----
    # folded fp8 tensors: [128, 4, C]; slot s = d // 128, d = s*128 + p
    hT2 = nc.dram_tensor("hT2", [P, 4, N], F8, kind="ExternalInput").ap()
    hqT2 = nc.dram_tensor("hqT2", [P, 4, NQ], F8, kind="ExternalInput").ap()
    wq2 = nc.dram_tensor("wq2", [P, 4, D], F8, kind="ExternalInput").ap()
    wk2 = nc.dram_tensor("wk2", [P, 4, D], F8, kind="ExternalInput").ap()
    wv2 = nc.dram_tensor("wv2", [P, 4, D], F8, kind="ExternalInput").ap()
    wo2 = nc.dram_tensor("wo2", [P, 4, D], F8, kind="ExternalInput").ap()
    maskT = nc.dram_tensor("maskT", [N, NQ], BF, kind="ExternalInput").ap()
    w1T = nc.dram_tensor("w1T", [D, DFF], BF, kind="ExternalInput").ap()
    w2T = nc.dram_tensor("w2T", [DFF, D], BF, kind="ExternalInput").ap()
    bq = nc.dram_tensor("bq", [D], F32, kind="ExternalInput").ap()
    b1 = nc.dram_tensor("b1", [DFF], F32, kind="ExternalInput").ap()
    b22 = nc.dram_tensor("b22", [1, D], BF, kind="ExternalInput").ap()
    lnc = nc.dram_tensor("lnc", [P, 4 * D], F32, kind="ExternalInput").ap()
    hq = nc.dram_tensor("hq", [NQ, D], F32, kind="ExternalInput").ap()
    identb = nc.dram_tensor("identb", [P, P], BF, kind="ExternalInput").ap()
    out = nc.dram_tensor("out", [NQ, D], F32, kind="ExternalOutput").ap()

    with tile.TileContext(nc) as tc:
        _emit(nc, tc, locals())
    nc.compile()
    return nc


def _emit(nc, tc, t):
    hT2, hqT2, maskT = t["hT2"], t["hqT2"], t["maskT"]
    wq2, wk2, wv2, wo2 = t["wq2"], t["wk2"], t["wv2"], t["wo2"]
    w1T, w2T = t["w1T"], t["w2T"]
    bq, b1, b22, lnc, hq = t["bq"], t["b1"], t["b22"], t["lnc"], t["hq"]
    identb, out = t["identb"], t["out"]

    from contextlib import ExitStack

    es = ExitStack()
    with es:
        cpool = es.enter_context(tc.tile_pool(name="const", bufs=1))
        h1pool = es.enter_context(tc.tile_pool(name="h1p", bufs=1))
        qkv_es = ExitStack()
        qkvpool = qkv_es.enter_context(tc.tile_pool(name="qkvp", bufs=1))
        mpool = qkv_es.enter_context(tc.tile_pool(name="maskp", bufs=1))
        proj_es = ExitStack()
        ppool = proj_es.enter_context(tc.tile_pool(name="projp", bufs=1))

        # ---- DMAs in first-use order ----
        wq_sb = ppool.tile([P, 4, D], F8, tag="wq")
        nc.sync.dma_start(wq_sb[:], wq2[:])
        hqT_sb = ppool.tile([P, 4, NQ], F8, tag="hqT")
        nc.sync.dma_start(hqT_sb[:], hqT2[:])
        bq_sb = cpool.tile([P, 4], F32, tag="bq")
        nc.sync.dma_start(bq_sb[:], bq.rearrange("(t p) -> p t", p=P))
        wk_sb = ppool.tile([P, 4, D], F8, tag="wk")
        nc.sync.dma_start(wk_sb[:], wk2[:])
        hT_sb = ppool.tile([P, 4, N], F8, tag="hT")
        nc.sync.dma_start(hT_sb[:, :, 0:N // 2], hT2[:, :, 0:N // 2])
        nc.sync.dma_start(hT_sb[:, :, N // 2:N], hT2[:, :, N // 2:N])
        wv_sb = ppool.tile([P, 4, D], F8, tag="wv")
        nc.sync.dma_start(wv_sb[:], wv2[:])
        # mask: [128, mt, NQ], two halves, on the gpsimd queue
        mask_sb = mpool.tile([P, 32 * NQ], BF, tag="mask")
        mask3 = mask_sb.rearrange("p (mt j) -> p mt j", j=NQ)
        msrc = maskT.rearrange("(mt p) j -> p mt j", p=P)
        nc.gpsimd.dma_start(mask3[:, 0:16, :], msrc[:, 0:16, :])
        nc.gpsimd.dma_start(mask3[:, 16:32, :], msrc[:, 16:32, :])
        wo_sb = cpool.tile([P, 4, D], F8, tag="wo")
        nc.sync.dma_start(wo_sb[:], wo2[:])
        ln_sb = cpool.tile([P, 4 * D], F32, tag="lnc")
        nc.sync.dma_start(ln_sb[:], lnc[:])
        hq_sb = cpool.tile([P, 4, D], F32, tag="hq")
        nc.sync.dma_start(hq_sb[:], hq.rearrange("(t p) d -> p t d", p=P))
        b1_sb = cpool.tile([P, 16], F32, tag="b1")
        nc.sync.dma_start(b1_sb[:], b1.rearrange("(t p) -> p t", p=P))
        b2_sb = cpool.tile([1, D], BF, tag="b2")
        nc.sync.dma_start(b2_sb[:], b22[:])
        identb_sb = cpool.tile([P, P], BF, tag="idb")
        nc.sync.dma_start(identb_sb[:], identb[:])

        ones_sb = cpool.tile([1, P], BF, tag="ones")
        nc.vector.memset(ones_sb[:], 1.0)
        eps_sb = cpool.tile([P, 1], F32, tag="eps")
        nc.vector.memset(eps_sb[:], EPS)

        g1l = ln_sb[:, 0:D]
        b1l = ln_sb[:, D:2 * D]
        g2l = ln_sb[:, 2 * D:3 * D]
        b2l = ln_sb[:, 3 * D:4 * D]

        # ---- persistent attention state ----
        kT_sb = [qkvpool.tile([P, N], BF, tag=f"kt{i}", name=f"kT{i}")
                 for i in range(4)]
        qT_sb = [qkvpool.tile([P, NQ], BF, tag=f"qt{i}", name=f"qT{i}")
                 for i in range(4)]
        # v_ext[mt]: [128, 8*65]; head h cols h*65..h*65+64, col 64 = ones
        v_sb = [qkvpool.tile([P, H * (DK + 1)], BF, tag=f"v{i}", name=f"v{i}")
                for i in range(32)]
        for mt in range(32):
            vv = v_sb[mt].rearrange("p (h c) -> p h c", c=DK + 1)
            nc.vector.memset(vv[:, :, DK:DK + 1], 1.0)
        # ctxT2[sp]: fp8 [128, 2, NQ] = 64*ctx_norm for head-pairs 2sp, 2sp+1
        ctxT2 = [h1pool.tile([P, 2 * NQ], F8, tag=f"cx{i}", name=f"ctxT2{i}")
                 for i in range(2)]
        h1acc = [h1pool.tile([P, D], F32, tag=f"ha{i}", name=f"h1acc{i}")
                 for i in range(4)]
        h1_sb = [h1pool.tile([P, D], F32, tag=f"h1_{i}", name=f"h1_{i}")
                 for i in range(4)]
        h1T_sb = [h1pool.tile([P, NQ], BF, tag=f"h1T{i}", name=f"h1T{i}")
                  for i in range(4)]

        # ================= projections (fp8 DoubleRow) =================
        with tc.tile_pool(name="psproj", bufs=4, space="PSUM") as psp:
            # qT[tt] = (Wq^T folded . hq)/32 + bq
            for tt in range(4):
                ps = psp.tile([P, NQ], F32, tag="pp", name="ps_q")
                for sp in range(2):
                    nc.tensor.matmul(ps[:],
                                     wq_sb[:, 2 * sp:2 * sp + 2,
                                           tt * P:(tt + 1) * P],
                                     hqT_sb[:, 2 * sp:2 * sp + 2, :],
                                     start=(sp == 0), stop=(sp == 1),
                                     perf_mode=DR)
                nc.scalar.activation(qT_sb[tt][:], ps[:], AF.Identity,
                                     bias=bq_sb[:, tt:tt + 1], scale=1.0 / WS)
            # kT[tt][:, c] (bk dropped: exact under softmax)
            for tt in range(4):
                for c in range(8):
                    ps = psp.tile([P, D], F32, tag="pp", name="ps_k")
                    for sp in range(2):
                        nc.tensor.matmul(ps[:],
                                         wk_sb[:, 2 * sp:2 * sp + 2,
                                               tt * P:(tt + 1) * P],
                                         hT_sb[:, 2 * sp:2 * sp + 2,
                                               c * D:(c + 1) * D],
                                         start=(sp == 0), stop=(sp == 1),
                                         perf_mode=DR)
                    nc.vector.tensor_scalar_mul(
                        kT_sb[tt][:, c * D:(c + 1) * D], ps[:], 1.0 / WS)
            # v[mc] (bv folded into host bo')
            for mc in range(32):
                ps = psp.tile([P, D], F32, tag="pp", name="ps_v")
                for sp in range(2):
                    nc.tensor.matmul(ps[:],
                                     hT_sb[:, 2 * sp:2 * sp + 2,
                                           mc * P:(mc + 1) * P],
                                     wv_sb[:, 2 * sp:2 * sp + 2, :],
                                     start=(sp == 0), stop=(sp == 1),
                                     perf_mode=DR)
                vv = v_sb[mc].rearrange("p (h c) -> p h c", c=DK + 1)
                nc.scalar.activation(vv[:, :, 0:DK],
                                     ps.rearrange("p (h c) -> p h c", c=DK),
                                     AF.Copy, scale=1.0 / WS)
        proj_es.close()

        if STOP_AT == 1:
            # anchor projections: out[qt] = f32(kT[qt][:, :512] + qT) + v
            for qt in range(4):
                cv = h1pool.tile([P, D], F32, tag="dbg", bufs=2, name="cv")
                nc.vector.tensor_add(cv[:], kT_sb[qt][:, 0:D], qT_sb[qt][:])
                nc.vector.tensor_add(cv[:], cv[:], v_sb[qt * 8][:, 0:D])
                nc.sync.dma_start(out[qt * P:(qt + 1) * P, :], cv[:])
            qkv_es.close()
            return

        # ================= attention =================
        with tc.tile_pool(name="attp", bufs=1) as apool, \
             tc.tile_pool(name="psatt", bufs=1, space="PSUM") as psa:
            for hp in range(4):
                ctx_ps = [psa.tile([P, NQ], F32, tag="pc", bufs=2,
                                   name="ctx_ps") for _ in range(2)]
                for g in range(16):
                    sp = [psa.tile([P, 1024], F32, tag="ps", bufs=2,
                                   name="sc_ps") for _ in range(2)]
                    at = [apool.tile([P, 1024], BF, tag="at", bufs=6,
                                     name="at") for _ in range(2)]
                    for i, po in ((0, 0), (1, DK)):
                        for j in range(2):
                            mt = 2 * g + j
                            nc.tensor.matmul(
                                sp[i][:, j * NQ:(j + 1) * NQ],
                                kT_sb[hp][po:po + DK, mt * P:(mt + 1) * P],
                                qT_sb[hp][po:po + DK, :],
                                start=True, stop=True)
                    for i in range(2):
                        nc.scalar.activation(at[i][:], sp[i][:], AF.Exp,
                                             scale=0.125)
                        nc.vector.tensor_mul(
                            at[i][:], at[i][:],
                            mask_sb[:, g * 1024:(g + 1) * 1024])
                    for i, h in ((0, 2 * hp), (1, 2 * hp + 1)):
                        for j in range(2):
                            mt = 2 * g + j
                            nc.tensor.matmul(
                                ctx_ps[i][0:DK + 1, :],
                                v_sb[mt][:, h * 65:h * 65 + 65],
                                at[i][:, j * NQ:(j + 1) * NQ],
                                start=(mt == 0), stop=(mt == 31))
                # normalize into fp8 ctxT2: 64 * ctx / rowsum
                dst = ctxT2[hp // 2]
                col = (hp % 2) * NQ
                for i, po in ((0, 0), (1, DK)):
                    rec = apool.tile([1, NQ], F32, tag="rec", bufs=2,
                                     name="rec")
                    nc.vector.reciprocal(rec[:], ctx_ps[i][DK:DK + 1, :])
                    bc = apool.tile([P, NQ], F32, tag="bc", bufs=2, name="bc")
                    nc.gpsimd.partition_broadcast(bc[:], rec[:])
                    nc.vector.scalar_tensor_tensor(
                        dst[po:po + DK, col:col + NQ], ctx_ps[i][0:DK, :],
                        CS, bc[0:DK, :], op0=ALU.mult, op1=ALU.mult)
                # stream Wo for the completed head-pair (fp8 DoubleRow)
                if STOP_AT == 2:
                    continue
                if hp % 2 == 1:
                    spx = hp // 2
                    src3 = ctxT2[spx].rearrange("p (i n) -> p i n", n=NQ)
                    for qt in range(4):
                        wops = psa.tile([P, D], F32, tag="wo", bufs=2,
                                        name="wo_ps")
                        nc.tensor.matmul(wops[:],
                                         src3[:, :, qt * P:(qt + 1) * P],
                                         wo_sb[:, 2 * spx:2 * spx + 2, :],
                                         start=True, stop=True, perf_mode=DR)
                        if hp == 1:
                            nc.vector.scalar_tensor_tensor(
                                h1acc[qt][:], wops[:], 1.0 / (WS * CS),
                                hq_sb[:, qt, :],
                                op0=ALU.mult, op1=ALU.add)
                        else:
                            nc.vector.scalar_tensor_tensor(
                                h1acc[qt][:], wops[:], 1.0 / (WS * CS),
                                h1acc[qt][:], op0=ALU.mult, op1=ALU.add)

        if STOP_AT == 2:
            # dump ctxT2 (fp8, 64*ctx_norm): out[qt] = ctxT2[qt//2][:, half]
            for qt in range(4):
                cv = h1pool.tile([P, D], F32, tag="dbg", bufs=2, name="cv")
                nc.vector.tensor_copy(
                    cv[:], ctxT2[qt // 2][:, (qt % 2) * NQ:(qt % 2 + 1) * NQ])
                nc.sync.dma_start(out[qt * P:(qt + 1) * P, :], cv[:])
            qkv_es.close()
            return
        if STOP_AT == 3:
            for qt in range(4):
                nc.sync.dma_start(out[qt * P:(qt + 1) * P, :], h1acc[qt][:])
            qkv_es.close()
            return

        # ---- LN1 + transpose (per query tile) ----
        with tc.tile_pool(name="pspost", bufs=2, space="PSUM") as psw:
            for qt in range(4):
                _layer_norm_sbuf(nc, h1pool, qt, h1acc[qt], g1l, b1l,
                                 h1_sb[qt], eps_sb)
                h1b = h1pool.tile([P, D], BF, tag="h1b", bufs=2,
                                  name="h1b")
                nc.vector.tensor_copy(h1b[:], h1_sb[qt][:])
                for i in range(4):
                    tp = psw.tile([P, P], BF, tag="tp", name="tp")
                    nc.tensor.transpose(tp[:], h1b[:, i * P:(i + 1) * P],
                                        identb_sb[:])
                    nc.vector.tensor_copy(
                        h1T_sb[i][:, qt * P:(qt + 1) * P], tp[:])

        qkv_es.close()

        # ================= FFN (ft-interleaved) =================
        with tc.tile_pool(name="ffnp", bufs=1) as fpool, \
             tc.tile_pool(name="psffn", bufs=1, space="PSUM") as psf:
            w1_sb = fpool.tile([P, 4, DFF], BF, tag="w1")
            nc.sync.dma_start(w1_sb[:],
                              w1T.rearrange("(t p) f -> p t f", p=P))
            w2_sb = fpool.tile([P, 16, D], BF, tag="w2")
            nc.sync.dma_start(w2_sb[:],
                              w2T.rearrange("(t p) d -> p t d", p=P))
            ff_ps = [psf.tile([P, D], F32, tag=f"fa{i}", name=f"ff_ps{i}")
                     for i in range(4)]
            for ft in range(16):
                ps = psf.tile([P, NQ], F32, tag="pf", bufs=2, name="f_ps")
                for s in range(4):
                    nc.tensor.matmul(ps[:],
                                     w1_sb[:, s, ft * P:(ft + 1) * P],
                                     h1T_sb[s][:], start=(s == 0),
                                     stop=(s == 3))
                fT = fpool.tile([P, NQ], BF, tag="fT", bufs=3, name="fT")
                nc.scalar.activation(fT[:], ps[:], AF.Relu,
                                     bias=b1_sb[:, ft:ft + 1])
                for qt in range(4):
                    nc.tensor.matmul(ff_ps[qt][:],
                                     fT[:, qt * P:(qt + 1) * P],
                                     w2_sb[:, ft, :], start=(ft == 0),
                                     stop=False)
            for qt in range(4):
                nc.tensor.matmul(ff_ps[qt][:], ones_sb[:], b2_sb[:],
                                 start=False, stop=True)
                h2 = _layer_norm_psum(nc, h1pool, qt, ff_ps[qt], h1_sb[qt],
                                      g2l, b2l, eps_sb)
                nc.sync.dma_start(out[qt * P:(qt + 1) * P, :], h2[:])


def _rstd(nc, pool, var, eps_sb):
    """rsqrt(var + eps) on DVE: bit-trick seed + 3 Newton steps."""
    I32 = mybir.dt.int32
    ve = pool.tile([P, 1], F32, tag="ve", bufs=4, name="ve")
    nc.vector.tensor_scalar_add(ve[:], var[:], eps_sb[:])
    rstd = pool.tile([P, 1], F32, tag="rstd", bufs=4, name="rstd")
    nc.vector.tensor_single_scalar(rstd[:].bitcast(I32), ve[:].bitcast(I32),
                                   1, op=ALU.arith_shift_right)
    nc.vector.tensor_single_scalar(rstd[:].bitcast(I32), rstd[:].bitcast(I32),
                                   0x5F3759DF, op=ALU.subtract)
    nc.vector.tensor_single_scalar(rstd[:].bitcast(I32), rstd[:].bitcast(I32),
                                   -1, op=ALU.mult)
    tq = pool.tile([P, 1], F32, tag="tq", bufs=4, name="tq")
    for _ in range(3):
        nc.vector.tensor_mul(tq[:], rstd[:], rstd[:])
        nc.vector.tensor_mul(tq[:], tq[:], ve[:])
        nc.vector.tensor_scalar_mul(tq[:], tq[:], -0.5)
        nc.vector.tensor_scalar_add(tq[:], tq[:], 1.5)
        nc.vector.tensor_mul(rstd[:], rstd[:], tq[:])
    return rstd


def _ln_core(nc, pool, x, s1, s2, g_sb, b_sb, out_tile, eps_sb):
    """out = normalize(x; s1=sum x, s2=sum x^2) * g + b. x modified."""
    nm = pool.tile([P, 1], F32, tag="nm", bufs=4, name="nm")
    nc.vector.tensor_scalar_mul(nm[:], s1[:], -1.0 / D)
    m2 = pool.tile([P, 1], F32, tag="m2", bufs=4, name="m2")
    nc.vector.tensor_mul(m2[:], nm[:], nm[:])
    var = pool.tile([P, 1], F32, tag="var", bufs=4, name="var")
    nc.vector.scalar_tensor_tensor(var[:], s2[:], 1.0 / D, m2[:],
                                   op0=ALU.mult, op1=ALU.subtract)
    rstd = _rstd(nc, pool, var, eps_sb)
    nc.vector.tensor_scalar_add(x[:], x[:], nm[:])
    nc.vector.tensor_scalar_mul(x[:], x[:], rstd[:])
    nc.vector.tensor_mul(out_tile[:], x[:], g_sb[:])
    nc.vector.tensor_add(out_tile[:], out_tile[:], b_sb[:])
    return out_tile


def _layer_norm_sbuf(nc, pool, uid, x_sb, g_sb, b_sb, out_tile, eps_sb):
    """out = LN(x_sb) * g + b; x_sb is f32 SBUF and is left modified."""
    s1 = pool.tile([P, 1], F32, tag="s1", bufs=4, name="s1")
    nc.vector.reduce_sum(s1[:], x_sb[:], axis=mybir.AxisListType.X)
    xsq = pool.tile([P, D], F32, tag="xsq", bufs=2, name="xsq")
    s2 = pool.tile([P, 1], F32, tag="s2", bufs=4, name="s2")
    nc.vector.tensor_mul(xsq[:], x_sb[:], x_sb[:])
    nc.vector.reduce_sum(s2[:], xsq[:], axis=mybir.AxisListType.X)
    return _ln_core(nc, pool, x_sb, s1, s2, g_sb, b_sb, out_tile, eps_sb)


def _layer_norm_psum(nc, pool, uid, z_ps, res_sb, g_sb, b_sb, eps_sb):
    """out = LN(z_ps + res_sb) * g + b, f32. Returns the output tile."""
    x = pool.tile([P, D], F32, tag="lx", bufs=2, name=f"x{uid}")
    s1 = pool.tile([P, 1], F32, tag="s1", bufs=4, name="s1")
    nc.vector.scalar_tensor_tensor(x[:], z_ps[:], 0.0, res_sb[:],
                                   op0=ALU.add, op1=ALU.add, accum_out=s1[:])
    xsq = pool.tile([P, D], F32, tag="xsq", bufs=2, name="xsq")
    s2 = pool.tile([P, 1], F32, tag="s2", bufs=4, name="s2")
    nc.vector.tensor_mul(xsq[:], x[:], x[:])
    nc.vector.reduce_sum(s2[:], xsq[:], axis=mybir.AxisListType.X)
    out_tile = pool.tile([P, D], F32, tag="h2", bufs=2, name=f"h2_{uid}")
    return _ln_core(nc, pool, x, s1, s2, g_sb, b_sb, out_tile, eps_sb)


def _fold(xT):
    """[512, C] -> [128, 4, C] with d = slot*128 + p."""
    c = xT.shape[1]
    return np.ascontiguousarray(
        xT.reshape(4, P, c).transpose(1, 0, 2))


def _prep_inputs(inputs):
    f32 = np.float32
    h = np.asarray(inputs["h"], f32)
    adj = np.asarray(inputs["adj"])

    def bf(x):
        return np.ascontiguousarray(np.asarray(x, f32).astype(BF16))

    def f8(x):
        return np.ascontiguousarray(np.asarray(x, f32).astype(FP8))

    hT = np.ascontiguousarray(h.T)
    adjb = (adj != 0)
    np.fill_diagonal(adjb, True)
    adjb_bf = adjb.astype(BF16)

    wq, wk, wv, wo = (np.asarray(inputs[k], f32)
                      for k in ("Wq", "Wk", "Wv", "Wo"))
    w1, w2 = np.asarray(inputs["W1"], f32), np.asarray(inputs["W2"], f32)
    bv = np.asarray(inputs["bv"], f32)
    bo = np.asarray(inputs["bo"], f32)
    bo2 = bo + bv @ wo.T  # bv folded through Wo

    lnc = np.concatenate([
        np.broadcast_to(np.asarray(inputs[k], f32), (P, D))
        for k in ("ln1_g", "ln1_b", "ln2_g", "ln2_b")], axis=1)

    shared = {
        "hT2": f8(_fold(hT)),
        "wq2": f8(_fold(wq.T) * WS), "wk2": f8(_fold(wk.T) * WS),
        "wv2": f8(_fold(wv.T) * WS), "wo2": f8(_fold(wo.T) * WS),
        "w1T": bf(w1.T), "w2T": bf(w2.T),
        "bq": np.ascontiguousarray(np.asarray(inputs["bq"], f32)),
        "b1": np.ascontiguousarray(np.asarray(inputs["b1"], f32)),
        "b22": bf(np.asarray(inputs["b2"], f32)[None, :]),
        "lnc": np.ascontiguousarray(lnc),
        "identb": np.eye(P, dtype=f32).astype(BF16),
    }
    in_maps = []
    for i in range(NCORES):
        r0 = i * NQ
        m = dict(shared)
        m["hqT2"] = f8(_fold(np.ascontiguousarray(hT[:, r0:r0 + NQ])))
        m["hq"] = np.ascontiguousarray(h[r0:r0 + NQ, :] + bo2)
        m["maskT"] = np.ascontiguousarray(adjb_bf[r0:r0 + NQ, :].T)
        in_maps.append(m)
    return in_maps


def kernel(**inputs) -> np.ndarray:
    global _CACHED, LAST_EXEC_NS
    if _CACHED is None:
        _CACHED = _build()
    nc = _CACHED
    in_maps = _prep_inputs(inputs)
    kw = {}
    if TRACE:
        kw = dict(trace=True, tmpdir=TRACE_DIR)
    res = run_bass_kernel_spmd(nc, in_maps, list(range(NCORES)), **kw)
    LAST_EXEC_NS = res.exec_time_ns
    return np.concatenate([res.results[i]["out"] for i in range(NCORES)],
                          axis=0)


# revision 14
# speedup vs baseline: 1.0635x; 1.0240x over previous
"""GraphTransformerLayer on 8 Trainium2 NeuronCores (Bass/Tile).

Sharding: 8-way along the query-node axis. Each core owns NQ=512 query rows,
computes full K/V projections (replicated), its slice of masked attention,
and its slice of the FFN. No collectives; the host concatenates the slices.

v3:
- Q/K/V and Wo matmuls in fp8e4 DoubleRow perf mode (2 k-subtiles per
  instruction, 0.5 cycles/row); weights host-folded to [128, 4, .], x32
  scaled (ctx x64) for fp8 range. bk dropped (exact under softmax), bv
  folded into host bo' = bo + bv @ Wo.T.
- Wo streams per head-pair into an SBUF f32 accumulator; ctx/wo PSUM tiles
  share one 4-deep ring so normalization lag never stalls the next pair.
- All DMA'd tensors host-packed to their exact SBUF layouts (contiguous,
  hardware-DGE friendly); issued in first-use order, w1/w2 before attention.
- LayerNorms use batched [128,4] stats + rstd across the 4 query tiles,
  ACT-engine center/scale, f32 PE transposes (no bf16 staging copy).
- FFN1/FFN2 interleaved per ft tile; per-qt LN2 + output DMA.
"""

import sys

if "/opt/trn_rl_repo" not in sys.path:
    sys.path.insert(0, "/opt/trn_rl_repo")

import numpy as np
import ml_dtypes

import concourse.bacc as bacc
import concourse.tile as tile
import concourse.mybir as mybir
from concourse.bass_utils import run_bass_kernel_spmd

BF16 = ml_dtypes.bfloat16
FP8 = ml_dtypes.float8_e4m3
F32 = mybir.dt.float32
BF = mybir.dt.bfloat16
F8 = mybir.dt.float8e4

N = 4096
D = 512
H = 8
DK = 64
DFF = 2048
NCORES = 8
NQ = N // NCORES
P = 128
EPS = 1e-5
WS = 32.0  # host weight pre-scale for fp8
CS = 64.0  # ctx pre-scale for fp8

ALU = mybir.AluOpType
AF = mybir.ActivationFunctionType
DR = mybir.MatmulPerfMode.DoubleRow

# set by test.py to capture a profile
TRACE = False
TRACE_DIR = None
LAST_EXEC_NS = None

# debug: truncate after a phase (1=proj, 2=attention ctx, 3=h1acc, 4=full)
STOP_AT = 4

_CACHED = None


def _build():
    nc = bacc.Bacc("TRN2", target_bir_lowering=False, debug=False,
                   num_devices=NCORES)

    # folded fp8 tensors: [128, 4, C]; d = slot*128 + p
    hT2 = nc.dram_tensor("hT2", [P, 4, N], F8, kind="ExternalInput").ap()
    hqT2 = nc.dram_tensor("hqT2", [P, 4, NQ], F8, kind="ExternalInput").ap()
    wq2 = nc.dram_tensor("wq2", [P, 4, D], F8, kind="ExternalInput").ap()
    wk2 = nc.dram_tensor("wk2", [P, 4, D], F8, kind="ExternalInput").ap()
    wv2 = nc.dram_tensor("wv2", [P, 4, D], F8, kind="ExternalInput").ap()
    wo2 = nc.dram_tensor("wo2", [P, 4, D], F8, kind="ExternalInput").ap()
    maskP = nc.dram_tensor("maskP", [P, 32, NQ], BF, kind="ExternalInput").ap()
    w1P = nc.dram_tensor("w1P", [P, 4, DFF], BF, kind="ExternalInput").ap()
    w2P = nc.dram_tensor("w2P", [P, 16, D], BF, kind="ExternalInput").ap()
    bqP = nc.dram_tensor("bqP", [P, 4], F32, kind="ExternalInput").ap()
    b1P = nc.dram_tensor("b1P", [P, 16], F32, kind="ExternalInput").ap()
    b22 = nc.dram_tensor("b22", [1, D], BF, kind="ExternalInput").ap()
    lnc = nc.dram_tensor("lnc", [P, 4 * D], F32, kind="ExternalInput").ap()
    hqP = nc.dram_tensor("hqP", [P, 4, D], F32, kind="ExternalInput").ap()
    identf = nc.dram_tensor("identf", [P, P], F32, kind="ExternalInput").ap()
    out = nc.dram_tensor("out", [NQ, D], F32, kind="ExternalOutput").ap()

    with tile.TileContext(nc) as tc:
        _emit(nc, tc, locals())
    nc.compile()
    return nc


def _emit(nc, tc, t):
    hT2, hqT2, maskP = t["hT2"], t["hqT2"], t["maskP"]
    wq2, wk2, wv2, wo2 = t["wq2"], t["wk2"], t["wv2"], t["wo2"]
    w1P, w2P = t["w1P"], t["w2P"]
    bqP, b1P, b22, lnc, hqP = t["bqP"], t["b1P"], t["b22"], t["lnc"], t["hqP"]
    identf, out = t["identf"], t["out"]

    from contextlib import ExitStack

    es = ExitStack()
    with es:
        cpool = es.enter_context(tc.tile_pool(name="const", bufs=1))
        h1pool = es.enter_context(tc.tile_pool(name="h1p", bufs=1))
        qkv_es = ExitStack()
        qkvpool = qkv_es.enter_context(tc.tile_pool(name="qkvp", bufs=1))
        mpool = qkv_es.enter_context(tc.tile_pool(name="maskp", bufs=1))
        proj_es = ExitStack()
        ppool = proj_es.enter_context(tc.tile_pool(name="projp", bufs=1))

        # ---- DMAs, first-use order, all contiguous host-packed ----
        wq_sb = ppool.tile([P, 4, D], F8, tag="wq")
        nc.sync.dma_start(wq_sb[:], wq2[:])
        hqT_sb = ppool.tile([P, 4, NQ], F8, tag="hqT")
        nc.sync.dma_start(hqT_sb[:], hqT2[:])
        wk_sb = ppool.tile([P, 4, D], F8, tag="wk")
        nc.sync.dma_start(wk_sb[:], wk2[:])
        hT_sb = ppool.tile([P, 4, N], F8, tag="hT")
        nc.sync.dma_start(hT_sb[:, :, 0:N // 2], hT2[:, :, 0:N // 2])
        nc.sync.dma_start(hT_sb[:, :, N // 2:N], hT2[:, :, N // 2:N])
        wv_sb = ppool.tile([P, 4, D], F8, tag="wv")
        nc.sync.dma_start(wv_sb[:], wv2[:])
        bq_sb = cpool.tile([P, 4], F32, tag="bq")
        nc.sync.dma_start(bq_sb[:], bqP[:])
        # mask in SBUF layout [128, mt, NQ], two halves on the gpsimd queue
        mask_sb = mpool.tile([P, 32, NQ], BF, tag="mask")
        nc.gpsimd.dma_start(mask_sb[:, 0:16, :], maskP[:, 0:16, :])
        nc.gpsimd.dma_start(mask_sb[:, 16:32, :], maskP[:, 16:32, :])
        wo_sb = cpool.tile([P, 4, D], F8, tag="wo")
        nc.sync.dma_start(wo_sb[:], wo2[:])
        lnab = cpool.tile([P, 4 * D], F32, tag="lnc")
        nc.sync.dma_start(lnab[:], lnc[:])
        hq_sb = cpool.tile([P, 4, D], F32, tag="hq")
        nc.sync.dma_start(hq_sb[:], hqP[:])
        b1_sb = cpool.tile([P, 16], F32, tag="b1")
        nc.sync.dma_start(b1_sb[:], b1P[:])
        b2_sb = cpool.tile([1, D], BF, tag="b2")
        nc.sync.dma_start(b2_sb[:], b22[:])
        identf_sb = cpool.tile([P, P], F32, tag="idf")
        nc.sync.dma_start(identf_sb[:], identf[:])
        ones_sb = cpool.tile([1, P], BF, tag="ones")
        nc.vector.memset(ones_sb[:], 1.0)
        eps_sb = cpool.tile([P, 1], F32, tag="eps")
        nc.vector.memset(eps_sb[:], EPS)

        g1l = lnab[:, 0:D]
        b1l = lnab[:, D:2 * D]
        g2l = lnab[:, 2 * D:3 * D]
        b2l = lnab[:, 3 * D:4 * D]

        # ---- persistent attention state ----
        kT_sb = [qkvpool.tile([P, N], BF, tag=f"kt{i}", name=f"kT{i}")
                 for i in range(4)]
        qT_sb = [qkvpool.tile([P, NQ], BF, tag=f"qt{i}", name=f"qT{i}")
                 for i in range(4)]
        v_sb = [qkvpool.tile([P, H * (DK + 1)], BF, tag=f"v{i}", name=f"v{i}")
                for i in range(32)]
        for mt in range(32):
            vv = v_sb[mt].rearrange("p (h c) -> p h c", c=DK + 1)
            nc.vector.memset(vv[:, :, DK:DK + 1], 1.0)
        ctxT2 = [h1pool.tile([P, 2 * NQ], F8, tag=f"cx{i}", name=f"ctxT2{i}")
                 for i in range(2)]
        h1acc = [h1pool.tile([P, D], F32, tag=f"ha{i}", name=f"h1acc{i}")
                 for i in range(4)]
        h1_sb = [h1pool.tile([P, D], F32, tag=f"h1_{i}", name=f"h1_{i}")
                 for i in range(4)]
        h1T_sb = [h1pool.tile([P, NQ], BF, tag=f"h1T{i}", name=f"h1T{i}")
                  for i in range(4)]

        # ================= projections (fp8 DoubleRow) =================
        with tc.tile_pool(name="psproj", bufs=4, space="PSUM") as psp:
            for tt in range(4):
                ps = psp.tile([P, NQ], F32, tag="pp", name="ps_q")
                for sp in range(2):
                    nc.tensor.matmul(ps[:],
                                     wq_sb[:, 2 * sp:2 * sp + 2,
                                           tt * P:(tt + 1) * P],
                                     hqT_sb[:, 2 * sp:2 * sp + 2, :],
                                     start=(sp == 0), stop=(sp == 1),
                                     perf_mode=DR)
                nc.scalar.activation(qT_sb[tt][:], ps[:], AF.Identity,
                                     bias=bq_sb[:, tt:tt + 1], scale=1.0 / WS)
            for tt in range(4):
                for c in range(8):
                    ps = psp.tile([P, D], F32, tag="pp", name="ps_k")
                    for sp in range(2):
                        nc.tensor.matmul(ps[:],
                                         wk_sb[:, 2 * sp:2 * sp + 2,
                                               tt * P:(tt + 1) * P],
                                         hT_sb[:, 2 * sp:2 * sp + 2,
                                               c * D:(c + 1) * D],
                                         start=(sp == 0), stop=(sp == 1),
                                         perf_mode=DR)
                    nc.vector.tensor_scalar_mul(
                        kT_sb[tt][:, c * D:(c + 1) * D], ps[:], 1.0 / WS)
            for mc in range(32):
                ps = psp.tile([P, D], F32, tag="pp", name="ps_v")
                for sp in range(2):
                    nc.tensor.matmul(ps[:],
                                     hT_sb[:, 2 * sp:2 * sp + 2,
                                           mc * P:(mc + 1) * P],
                                     wv_sb[:, 2 * sp:2 * sp + 2, :],
                                     start=(sp == 0), stop=(sp == 1),
                                     perf_mode=DR)
                vv = v_sb[mc].rearrange("p (h c) -> p h c", c=DK + 1)
                nc.scalar.activation(vv[:, :, 0:DK],
                                     ps.rearrange("p (h c) -> p h c", c=DK),
                                     AF.Copy, scale=1.0 / WS)
        proj_es.close()

        if STOP_AT == 1:
            for qt in range(4):
                cv = h1pool.tile([P, D], F32, tag="dbg", bufs=2, name="cv")
                nc.vector.tensor_add(cv[:], kT_sb[qt][:, 0:D], qT_sb[qt][:])
                nc.vector.tensor_add(cv[:], cv[:], v_sb[qt * 8][:, 0:D])
                nc.sync.dma_start(out[qt * P:(qt + 1) * P, :], cv[:])
            qkv_es.close()
            return

        # ================= attention =================
        with tc.tile_pool(name="attp", bufs=1) as apool, \
             tc.tile_pool(name="psatt", bufs=1, space="PSUM") as psa:
            for hp in range(4):
                ctx_ps = [psa.tile([P, NQ], F32, tag="pc", bufs=4,
                                   name="ctx_ps") for _ in range(2)]
                for g in range(16):
                    sp = [psa.tile([P, 1024], F32, tag="ps", bufs=2,
                                   name="sc_ps") for _ in range(2)]
                    at = [apool.tile([P, 1024], BF, tag="at", bufs=6,
                                     name="at") for _ in range(2)]
                    for i, po in ((0, 0), (1, DK)):
                        for j in range(2):
                            mt = 2 * g + j
                            nc.tensor.matmul(
                                sp[i][:, j * NQ:(j + 1) * NQ],
                                kT_sb[hp][po:po + DK, mt * P:(mt + 1) * P],
                                qT_sb[hp][po:po + DK, :],
                                start=True, stop=True)
                    for i in range(2):
                        nc.scalar.activation(at[i][:], sp[i][:], AF.Exp,
                                             scale=0.125)
                        nc.vector.tensor_mul(
                            at[i][:], at[i][:],
                            mask_sb[:, 2 * g:2 * g + 2, :])
                    for i, h in ((0, 2 * hp), (1, 2 * hp + 1)):
                        for j in range(2):
                            mt = 2 * g + j
                            nc.tensor.matmul(
                                ctx_ps[i][0:DK + 1, :],
                                v_sb[mt][:, h * 65:h * 65 + 65],
                                at[i][:, j * NQ:(j + 1) * NQ],
                                start=(mt == 0), stop=(mt == 31))
                # normalize into fp8 ctxT2: 64 * ctx / rowsum
                dst = ctxT2[hp // 2]
                col = (hp % 2) * NQ
                for i, po in ((0, 0), (1, DK)):
                    rec = apool.tile([1, NQ], F32, tag="rec", bufs=2,
                                     name="rec")
                    nc.vector.reciprocal(rec[:], ctx_ps[i][DK:DK + 1, :])
                    bc = apool.tile([P, NQ], F32, tag="bc", bufs=2, name="bc")
                    nc.gpsimd.partition_broadcast(bc[:], rec[:])
                    nc.vector.scalar_tensor_tensor(
                        dst[po:po + DK, col:col + NQ], ctx_ps[i][0:DK, :],
                        CS, bc[0:DK, :], op0=ALU.mult, op1=ALU.mult)
                if STOP_AT == 2:
                    continue
                # stream Wo for the completed head-pair (fp8 DoubleRow)
                if hp % 2 == 1:
                    spx = hp // 2
                    src3 = ctxT2[spx].rearrange("p (i n) -> p i n", n=NQ)
                    for qt in range(4):
                        wops = psa.tile([P, D], F32, tag="pc", bufs=4,
                                        name="wo_ps")
                        nc.tensor.matmul(wops[:],
                                         src3[:, :, qt * P:(qt + 1) * P],
                                         wo_sb[:, 2 * spx:2 * spx + 2, :],
                                         start=True, stop=True, perf_mode=DR)
                        if hp == 1:
                            nc.vector.scalar_tensor_tensor(
                                h1acc[qt][:], wops[:], 1.0 / (WS * CS),
                                hq_sb[:, qt:qt + 1, :], op0=ALU.mult, op1=ALU.add)
                        else:
                            nc.vector.scalar_tensor_tensor(
                                h1acc[qt][:], wops[:], 1.0 / (WS * CS),
                                h1acc[qt][:], op0=ALU.mult, op1=ALU.add)

        if STOP_AT == 2:
            for qt in range(4):
                cv = h1pool.tile([P, D], F32, tag="dbg", bufs=2, name="cv")
                nc.vector.tensor_copy(
                    cv[:], ctxT2[qt // 2][:, (qt % 2) * NQ:(qt % 2 + 1) * NQ])
                nc.sync.dma_start(out[qt * P:(qt + 1) * P, :], cv[:])
            qkv_es.close()
            return
        if STOP_AT == 3:
            for qt in range(4):
                nc.sync.dma_start(out[qt * P:(qt + 1) * P, :], h1acc[qt][:])
            qkv_es.close()
            return

        # ---- LN1 (batched stats) + f32 transpose ----
        with tc.tile_pool(name="pspost", bufs=2, space="PSUM") as psw:
            s1 = h1pool.tile([P, 4], F32, tag="s1a", name="s1a")
            s2 = h1pool.tile([P, 4], F32, tag="s2a", name="s2a")
            for qt in range(4):
                nc.vector.reduce_sum(s1[:, qt:qt + 1], h1acc[qt][:],
                                     axis=mybir.AxisListType.X)
                xsq = h1pool.tile([P, D], F32, tag="xsq", bufs=2, name="xsq")
                nc.vector.tensor_mul(xsq[:], h1acc[qt][:], h1acc[qt][:])
                nc.vector.reduce_sum(s2[:, qt:qt + 1], xsq[:],
                                     axis=mybir.AxisListType.X)
            rstd4, nmr4 = _stats4(nc, h1pool, s1, s2, eps_sb, "a")
            for qt in range(4):
                xn = h1pool.tile([P, D], F32, tag="xn", bufs=2, name="xn")
                nc.scalar.activation(xn[:], h1acc[qt][:], AF.Identity,
                                     bias=nmr4[:, qt:qt + 1],
                                     scale=rstd4[:, qt:qt + 1])
                nc.vector.tensor_mul(h1_sb[qt][:], xn[:], g1l)
                nc.vector.tensor_add(h1_sb[qt][:], h1_sb[qt][:], b1l)
                for i in range(4):
                    tp = psw.tile([P, P], F32, tag="tp", name="tp")
                    nc.tensor.transpose(tp[:], h1_sb[qt][:, i * P:(i + 1) * P],
                                        identf_sb[:])
                    nc.vector.tensor_copy(
                        h1T_sb[i][:, qt * P:(qt + 1) * P], tp[:])

        qkv_es.close()

        # ================= FFN (ft-interleaved) =================
        ffnp = es.enter_context(tc.tile_pool(name="ffnp", bufs=1))
        w1_sb = ffnp.tile([P, 4, DFF], BF, tag="w1")
        nc.sync.dma_start(w1_sb[:], w1P[:])
        w2_sb = ffnp.tile([P, 16, D], BF, tag="w2")
        nc.sync.dma_start(w2_sb[:], w2P[:])
        with tc.tile_pool(name="psffn", bufs=1, space="PSUM") as psf:
            ff_ps = [psf.tile([P, D], F32, tag=f"fa{i}", name=f"ff_ps{i}")
                     for i in range(4)]
            for ft in range(16):
                ps = psf.tile([P, NQ], F32, tag="pf", bufs=2, name="f_ps")
                for s in range(4):
                    nc.tensor.matmul(ps[:],
                                     w1_sb[:, s:s + 1, ft * P:(ft + 1) * P],
                                     h1T_sb[s][:], start=(s == 0),
                                     stop=(s == 3))
                fT = ffnp.tile([P, NQ], BF, tag="fT", bufs=3, name="fT")
                nc.scalar.activation(fT[:], ps[:], AF.Relu,
                                     bias=b1_sb[:, ft:ft + 1])
                for qt in range(4):
                    nc.tensor.matmul(ff_ps[qt][:],
                                     fT[:, qt * P:(qt + 1) * P],
                                     w2_sb[:, ft:ft + 1, :], start=(ft == 0),
                                     stop=False)
            # ---- +b2, then LN2 with batched stats ----
            s1 = h1pool.tile([P, 4], F32, tag="s1b", name="s1b")
            s2 = h1pool.tile([P, 4], F32, tag="s2b", name="s2b")
            x2 = []
            for qt in range(4):
                nc.tensor.matmul(ff_ps[qt][:], ones_sb[:], b2_sb[:],
                                 start=False, stop=True)
                x = h1pool.tile([P, D], F32, tag=f"x2{qt}", name=f"x2{qt}")
                nc.vector.scalar_tensor_tensor(x[:], ff_ps[qt][:], 0.0,
                                               h1_sb[qt][:], op0=ALU.add,
                                               op1=ALU.add,
                                               accum_out=s1[:, qt:qt + 1])
                xsq = h1pool.tile([P, D], F32, tag="xsq", bufs=2, name="xsq")
                nc.vector.tensor_mul(xsq[:], x[:], x[:])
                nc.vector.reduce_sum(s2[:, qt:qt + 1], xsq[:],
                                     axis=mybir.AxisListType.X)
                x2.append(x)
            rstd4, nmr4 = _stats4(nc, h1pool, s1, s2, eps_sb, "b")
            for qt in range(4):
                xn = h1pool.tile([P, D], F32, tag="xn", bufs=2, name="xn")
                nc.scalar.activation(xn[:], x2[qt][:], AF.Identity,
                                     bias=nmr4[:, qt:qt + 1],
                                     scale=rstd4[:, qt:qt + 1])
                h2 = h1pool.tile([P, D], F32, tag="h2o", bufs=2, name="h2")
                nc.vector.tensor_mul(h2[:], xn[:], g2l)
                nc.vector.tensor_add(h2[:], h2[:], b2l)
                nc.sync.dma_start(out[qt * P:(qt + 1) * P, :], h2[:])


def _stats4(nc, pool, s1, s2, eps_sb, uid):
    """Batched LN stats: from per-qt sums s1,s2 [P,4] compute rstd4 and
    nmr4 = (-mean * rstd) [P,4]."""
    I32 = mybir.dt.int32
    nm = pool.tile([P, 4], F32, tag="nm4", bufs=2, name=f"nm4{uid}")
    nc.vector.tensor_scalar_mul(nm[:], s1[:], -1.0 / D)
    m2 = pool.tile([P, 4], F32, tag="m24", bufs=2, name=f"m24{uid}")
    nc.vector.tensor_mul(m2[:], nm[:], nm[:])
    var = pool.tile([P, 4], F32, tag="var4", bufs=2, name=f"var4{uid}")
    nc.vector.scalar_tensor_tensor(var[:], s2[:], 1.0 / D, m2[:],
                                   op0=ALU.mult, op1=ALU.subtract)
    ve = pool.tile([P, 4], F32, tag="ve4", bufs=2, name=f"ve4{uid}")
    nc.vector.tensor_scalar_add(ve[:], var[:], eps_sb[:])
    rstd = pool.tile([P, 4], F32, tag="rs4", bufs=2, name=f"rs4{uid}")
    nc.vector.tensor_single_scalar(rstd[:].bitcast(I32), ve[:].bitcast(I32),
                                   1, op=ALU.arith_shift_right)
    nc.vector.tensor_single_scalar(rstd[:].bitcast(I32), rstd[:].bitcast(I32),
                                   0x5F3759DF, op=ALU.subtract)
    nc.vector.tensor_single_scalar(rstd[:].bitcast(I32), rstd[:].bitcast(I32),
                                   -1, op=ALU.mult)
    tq = pool.tile([P, 4], F32, tag="tq4", bufs=2, name=f"tq4{uid}")
    for _ in range(3):
        nc.vector.tensor_mul(tq[:], rstd[:], rstd[:])
        nc.vector.tensor_mul(tq[:], tq[:], ve[:])
        nc.vector.tensor_scalar_mul(tq[:], tq[:], -0.5)
        nc.vector.tensor_scalar_add(tq[:], tq[:], 1.5)
        nc.vector.tensor_mul(rstd[:], rstd[:], tq[:])
    nmr = pool.tile([P, 4], F32, tag="nmr4", bufs=2, name=f"nmr4{uid}")
    nc.vector.tensor_mul(nmr[:], nm[:], rstd[:])
    return rstd, nmr


def _fold(xT):
    """[512, C] -> [128, 4, C] with d = slot*128 + p."""
    c = xT.shape[1]
    return np.ascontiguousarray(xT.reshape(4, P, c).transpose(1, 0, 2))


def _prep_inputs(inputs):
    f32 = np.float32
    h = np.asarray(inputs["h"], f32)
    adj = np.asarray(inputs["adj"])

    def bf(x):
        return np.ascontiguousarray(np.asarray(x, f32).astype(BF16))

    def f8(x):
        return np.ascontiguousarray(np.asarray(x, f32).astype(FP8))

    hT = np.ascontiguousarray(h.T)
    adjb = (adj != 0)
    np.fill_diagonal(adjb, True)
    adjb_bf = adjb.astype(BF16)

    wq, wk, wv, wo = (np.asarray(inputs[k], f32)
                      for k in ("Wq", "Wk", "Wv", "Wo"))
    w1, w2 = np.asarray(inputs["W1"], f32), np.asarray(inputs["W2"], f32)
    bv = np.asarray(inputs["bv"], f32)
    bo = np.asarray(inputs["bo"], f32)
    bo2 = bo + bv @ wo.T  # bv folded through Wo

    lnc = np.concatenate([
        np.broadcast_to(np.asarray(inputs[k], f32), (P, D))
        for k in ("ln1_g", "ln1_b", "ln2_g", "ln2_b")], axis=1)

    shared = {
        "hT2": f8(_fold(hT)),
        "wq2": f8(_fold(wq.T) * WS), "wk2": f8(_fold(wk.T) * WS),
        "wv2": f8(_fold(wv.T) * WS), "wo2": f8(_fold(wo.T) * WS),
        "w1P": bf(w1.T.reshape(4, P, DFF).transpose(1, 0, 2)),
        "w2P": bf(w2.T.reshape(16, P, D).transpose(1, 0, 2)),
        "bqP": np.ascontiguousarray(
            np.asarray(inputs["bq"], f32).reshape(4, P).T),
        "b1P": np.ascontiguousarray(
            np.asarray(inputs["b1"], f32).reshape(16, P).T),
        "b22": bf(np.asarray(inputs["b2"], f32)[None, :]),
        "lnc": np.ascontiguousarray(lnc),
        "identf": np.eye(P, dtype=f32),
    }
    in_maps = []
    for i in range(NCORES):
        r0 = i * NQ
        m = dict(shared)
        m["hqT2"] = f8(_fold(np.ascontiguousarray(hT[:, r0:r0 + NQ])))
        m["hqP"] = np.ascontiguousarray(
            (h[r0:r0 + NQ, :] + bo2).reshape(4, P, D).transpose(1, 0, 2))
        m["maskP"] = np.ascontiguousarray(
            adjb_bf[r0:r0 + NQ, :].T.reshape(32, P, NQ).transpose(1, 0, 2))
        in_maps.append(m)
    return in_maps


def kernel(**inputs) -> np.ndarray:
    global _CACHED, LAST_EXEC_NS
    if _CACHED is None:
        _CACHED = _build()
    nc = _CACHED
    in_maps = _prep_inputs(inputs)
    kw = {}
    if TRACE:
        kw = dict(trace=True, tmpdir=TRACE_DIR)
    res = run_bass_kernel_spmd(nc, in_maps, list(range(NCORES)), **kw)
    LAST_EXEC_NS = res.exec_time_ns
    return np.concatenate([res.results[i]["out"] for i in range(NCORES)],
                          axis=0)


# revision 15
# speedup vs baseline: 1.1576x; 1.0885x over previous
"""GraphTransformerLayer on 8 Trainium2 NeuronCores (Bass/Tile).

Sharding: 8-way along the query-node axis. Each core owns NQ=512 query rows,
computes full K/V projections (replicated), its slice of masked attention,
and its slice of the FFN. No collectives; the host concatenates the slices.

v3:
- Q/K/V and Wo matmuls in fp8e4 DoubleRow perf mode (2 k-subtiles per
  instruction, 0.5 cycles/row); weights host-folded to [128, 4, .], x32
  scaled (ctx x64) for fp8 range. bk dropped (exact under softmax), bv
  folded into host bo' = bo + bv @ Wo.T.
- Wo streams per head-pair into an SBUF f32 accumulator; ctx/wo PSUM tiles
  share one 4-deep ring so normalization lag never stalls the next pair.
- All DMA'd tensors host-packed to their exact SBUF layouts (contiguous,
  hardware-DGE friendly); issued in first-use order, w1/w2 before attention.
- LayerNorms use batched [128,4] stats + rstd across the 4 query tiles,
  ACT-engine center/scale, f32 PE transposes (no bf16 staging copy).
- FFN1/FFN2 interleaved per ft tile; per-qt LN2 + output DMA.
"""

import sys

if "/opt/trn_rl_repo" not in sys.path:
    sys.path.insert(0, "/opt/trn_rl_repo")

import numpy as np
import ml_dtypes

import concourse.bacc as bacc
import concourse.tile as tile
import concourse.mybir as mybir
from concourse.bass_utils import run_bass_kernel_spmd

BF16 = ml_dtypes.bfloat16
FP8 = ml_dtypes.float8_e4m3
F32 = mybir.dt.float32
BF = mybir.dt.bfloat16
F8 = mybir.dt.float8e4

N = 4096
D = 512
H = 8
DK = 64
DFF = 2048
NCORES = 8
NQ = N // NCORES
P = 128
EPS = 1e-5
WS = 32.0  # host weight pre-scale for fp8
CS = 64.0  # ctx pre-scale for fp8

ALU = mybir.AluOpType
AF = mybir.ActivationFunctionType
DR = mybir.MatmulPerfMode.DoubleRow

# set by test.py to capture a profile
TRACE = False
TRACE_DIR = None
LAST_EXEC_NS = None

# debug: truncate after a phase (1=proj, 2=attention ctx, 3=h1acc, 4=full)
STOP_AT = 4

_CACHED = None


def _build():
    nc = bacc.Bacc("TRN2", target_bir_lowering=False, debug=False,
                   num_devices=NCORES)

    # folded fp8 tensors: [128, 4, C]; d = slot*128 + p
    hT2 = nc.dram_tensor("hT2", [P, 4, N], F8, kind="ExternalInput").ap()
    hqT2 = nc.dram_tensor("hqT2", [P, 4, NQ], F8, kind="ExternalInput").ap()
    wq2 = nc.dram_tensor("wq2", [P, 4, D], F8, kind="ExternalInput").ap()
    wk2 = nc.dram_tensor("wk2", [P, 4, D], F8, kind="ExternalInput").ap()
    wv2 = nc.dram_tensor("wv2", [P, 4, D], F8, kind="ExternalInput").ap()
    wo2 = nc.dram_tensor("wo2", [P, 4, D], F8, kind="ExternalInput").ap()
    maskP = nc.dram_tensor("maskP", [P, 32, NQ], BF, kind="ExternalInput").ap()
    w1P = nc.dram_tensor("w1P", [P, 4, DFF], BF, kind="ExternalInput").ap()
    w2P = nc.dram_tensor("w2P", [P, 16, D], BF, kind="ExternalInput").ap()
    bqP = nc.dram_tensor("bqP", [P, 4], F32, kind="ExternalInput").ap()
    b1P = nc.dram_tensor("b1P", [P, 16], F32, kind="ExternalInput").ap()
    b22 = nc.dram_tensor("b22", [1, D], BF, kind="ExternalInput").ap()
    lnc = nc.dram_tensor("lnc", [P, 4 * D], F32, kind="ExternalInput").ap()
    hqP = nc.dram_tensor("hqP", [P, 4, D], F32, kind="ExternalInput").ap()
    identf = nc.dram_tensor("identf", [P, P], F32, kind="ExternalInput").ap()
    out = nc.dram_tensor("out", [NQ, D], F32, kind="ExternalOutput").ap()

    with tile.TileContext(nc) as tc:
        _emit(nc, tc, locals())
    nc.compile()
    return nc


def _emit(nc, tc, t):
    hT2, hqT2, maskP = t["hT2"], t["hqT2"], t["maskP"]
    wq2, wk2, wv2, wo2 = t["wq2"], t["wk2"], t["wv2"], t["wo2"]
    w1P, w2P = t["w1P"], t["w2P"]
    bqP, b1P, b22, lnc, hqP = t["bqP"], t["b1P"], t["b22"], t["lnc"], t["hqP"]
    identf, out = t["identf"], t["out"]

    from contextlib import ExitStack

    es = ExitStack()
    with es:
        cpool = es.enter_context(tc.tile_pool(name="const", bufs=1))
        h1pool = es.enter_context(tc.tile_pool(name="h1p", bufs=1))
        qkv_es = ExitStack()
        qkvpool = qkv_es.enter_context(tc.tile_pool(name="qkvp", bufs=1))
        mpool = qkv_es.enter_context(tc.tile_pool(name="maskp", bufs=1))
        proj_es = ExitStack()
        ppool = proj_es.enter_context(tc.tile_pool(name="projp", bufs=1))

        # ---- DMAs, first-use order, all contiguous host-packed ----
        wq_sb = ppool.tile([P, 4, D], F8, tag="wq")
        nc.sync.dma_start(wq_sb[:], wq2[:])
        hqT_sb = ppool.tile([P, 4, NQ], F8, tag="hqT")
        nc.sync.dma_start(hqT_sb[:], hqT2[:])
        wk_sb = ppool.tile([P, 4, D], F8, tag="wk")
        nc.sync.dma_start(wk_sb[:], wk2[:])
        hT_sb = ppool.tile([P, 4, N], F8, tag="hT")
        for ck in range(4):
            nc.sync.dma_start(hT_sb[:, :, ck * (N // 4):(ck + 1) * (N // 4)],
                              hT2[:, :, ck * (N // 4):(ck + 1) * (N // 4)])
        wv_sb = ppool.tile([P, 4, D], F8, tag="wv")
        nc.sync.dma_start(wv_sb[:], wv2[:])
        bq_sb = cpool.tile([P, 4], F32, tag="bq")
        nc.sync.dma_start(bq_sb[:], bqP[:])
        # mask in SBUF layout [128, mt, NQ], two halves on the gpsimd queue
        mask_sb = mpool.tile([P, 32, NQ], BF, tag="mask")
        nc.gpsimd.dma_start(mask_sb[:, 0:16, :], maskP[:, 0:16, :])
        nc.gpsimd.dma_start(mask_sb[:, 16:32, :], maskP[:, 16:32, :])
        wo_sb = cpool.tile([P, 4, D], F8, tag="wo")
        nc.sync.dma_start(wo_sb[:], wo2[:])
        lnab = cpool.tile([P, 4 * D], F32, tag="lnc")
        nc.sync.dma_start(lnab[:], lnc[:])
        hq_sb = cpool.tile([P, 4, D], F32, tag="hq")
        nc.sync.dma_start(hq_sb[:], hqP[:])
        b1_sb = cpool.tile([P, 16], F32, tag="b1")
        nc.sync.dma_start(b1_sb[:], b1P[:])
        b2_sb = cpool.tile([1, D], BF, tag="b2")
        nc.sync.dma_start(b2_sb[:], b22[:])
        identf_sb = cpool.tile([P, P], F32, tag="idf")
        nc.sync.dma_start(identf_sb[:], identf[:])
        ones_sb = cpool.tile([1, P], BF, tag="ones")
        nc.vector.memset(ones_sb[:], 1.0)
        eps_sb = cpool.tile([P, 1], F32, tag="eps")
        nc.vector.memset(eps_sb[:], EPS)

        g1l = lnab[:, 0:D]
        b1l = lnab[:, D:2 * D]
        g2l = lnab[:, 2 * D:3 * D]
        b2l = lnab[:, 3 * D:4 * D]

        # ---- persistent attention state ----
        kT_sb = [qkvpool.tile([P, N], BF, tag=f"kt{i}", name=f"kT{i}")
                 for i in range(4)]
        qT_sb = [qkvpool.tile([P, NQ], BF, tag=f"qt{i}", name=f"qT{i}")
                 for i in range(4)]
        v_sb = [qkvpool.tile([P, H * (DK + 1)], BF, tag=f"v{i}", name=f"v{i}")
                for i in range(32)]
        for mt in range(32):
            vv = v_sb[mt].rearrange("p (h c) -> p h c", c=DK + 1)
            nc.vector.memset(vv[:, :, DK:DK + 1], 1.0)
        ctxT2 = [h1pool.tile([P, 2 * NQ], F8, tag=f"cx{i}", name=f"ctxT2{i}")
                 for i in range(2)]
        h1acc = [h1pool.tile([P, D], F32, tag=f"ha{i}", name=f"h1acc{i}")
                 for i in range(4)]
        h1_sb = [h1pool.tile([P, D], F32, tag=f"h1_{i}", name=f"h1_{i}")
                 for i in range(4)]
        h1T_sb = [h1pool.tile([P, NQ], BF, tag=f"h1T{i}", name=f"h1T{i}")
                  for i in range(4)]

        # ================= projections (fp8 DoubleRow) =================
        with tc.tile_pool(name="psproj", bufs=4, space="PSUM") as psp:
            for tt in range(4):
                ps = psp.tile([P, NQ], F32, tag="pp", name="ps_q")
                for sp in range(2):
                    nc.tensor.matmul(ps[:],
                                     wq_sb[:, 2 * sp:2 * sp + 2,
                                           tt * P:(tt + 1) * P],
                                     hqT_sb[:, 2 * sp:2 * sp + 2, :],
                                     start=(sp == 0), stop=(sp == 1),
                                     perf_mode=DR)
                nc.scalar.activation(qT_sb[tt][:], ps[:], AF.Identity,
                                     bias=bq_sb[:, tt:tt + 1], scale=1.0 / WS)
            for tt in range(4):
                for c in range(8):
                    ps = psp.tile([P, D], F32, tag="pp", name="ps_k")
                    for sp in range(2):
                        nc.tensor.matmul(ps[:],
                                         wk_sb[:, 2 * sp:2 * sp + 2,
                                               tt * P:(tt + 1) * P],
                                         hT_sb[:, 2 * sp:2 * sp + 2,
                                               c * D:(c + 1) * D],
                                         start=(sp == 0), stop=(sp == 1),
                                         perf_mode=DR)
                    nc.vector.tensor_scalar_mul(
                        kT_sb[tt][:, c * D:(c + 1) * D], ps[:], 1.0 / WS)
            for mc in range(32):
                ps = psp.tile([P, D], F32, tag="pp", name="ps_v")
                for sp in range(2):
                    nc.tensor.matmul(ps[:],
                                     hT_sb[:, 2 * sp:2 * sp + 2,
                                           mc * P:(mc + 1) * P],
                                     wv_sb[:, 2 * sp:2 * sp + 2, :],
                                     start=(sp == 0), stop=(sp == 1),
                                     perf_mode=DR)
                vv = v_sb[mc].rearrange("p (h c) -> p h c", c=DK + 1)
                nc.scalar.activation(vv[:, :, 0:DK],
                                     ps.rearrange("p (h c) -> p h c", c=DK),
                                     AF.Copy, scale=1.0 / WS)
        proj_es.close()

        if STOP_AT == 1:
            for qt in range(4):
                cv = h1pool.tile([P, D], F32, tag="dbg", bufs=2, name="cv")
                nc.vector.tensor_add(cv[:], kT_sb[qt][:, 0:D], qT_sb[qt][:])
                nc.vector.tensor_add(cv[:], cv[:], v_sb[qt * 8][:, 0:D])
                nc.sync.dma_start(out[qt * P:(qt + 1) * P, :], cv[:])
            qkv_es.close()
            return

        # ================= attention =================
        with tc.tile_pool(name="attp", bufs=1) as apool, \
             tc.tile_pool(name="psatt", bufs=1, space="PSUM") as psa:
            for hp in range(4):
                ctx_ps = psa.tile([P, 2 * NQ], F32, tag="pc", bufs=1,
                                  name="ctx_ps")
                for g in range(16):
                    sp = [psa.tile([P, 1024], F32, tag="ps", bufs=3,
                                   name="sc_ps") for _ in range(2)]
                    at = [apool.tile([P, 1024], BF, tag="at", bufs=6,
                                     name="at") for _ in range(2)]
                    for i, po in ((0, 0), (1, DK)):
                        for j in range(2):
                            mt = 2 * g + j
                            nc.tensor.matmul(
                                sp[i][:, j * NQ:(j + 1) * NQ],
                                kT_sb[hp][po:po + DK, mt * P:(mt + 1) * P],
                                qT_sb[hp][po:po + DK, :],
                                start=True, stop=True)
                    for i in range(2):
                        nc.scalar.activation(at[i][:], sp[i][:], AF.Exp,
                                             scale=0.125)
                        nc.vector.tensor_mul(
                            at[i][:], at[i][:],
                            mask_sb[:, 2 * g:2 * g + 2, :])
                    for i, h in ((0, 2 * hp), (1, 2 * hp + 1)):
                        for j in range(2):
                            mt = 2 * g + j
                            nc.tensor.matmul(
                                ctx_ps[0:DK + 1, i * NQ:(i + 1) * NQ],
                                v_sb[mt][:, h * 65:h * 65 + 65],
                                at[i][:, j * NQ:(j + 1) * NQ],
                                start=(mt == 0), stop=(mt == 31))
                # normalize into fp8 ctxT2: 64 * ctx / rowsum
                dst = ctxT2[hp // 2]
                col = (hp % 2) * NQ
                for i, po in ((0, 0), (1, DK)):
                    rec = apool.tile([1, NQ], F32, tag="rec", bufs=2,
                                     name="rec")
                    nc.vector.reciprocal(
                        rec[:], ctx_ps[DK:DK + 1, i * NQ:(i + 1) * NQ])
                    bc = apool.tile([P, NQ], F32, tag="bc", bufs=2, name="bc")
                    nc.gpsimd.partition_broadcast(bc[:], rec[:])
                    nc.vector.scalar_tensor_tensor(
                        dst[po:po + DK, col:col + NQ],
                        ctx_ps[0:DK, i * NQ:(i + 1) * NQ],
                        CS, bc[0:DK, :], op0=ALU.mult, op1=ALU.mult)
                if STOP_AT == 2:
                    continue
                # stream Wo for the completed head-pair (fp8 DoubleRow)
                if hp % 2 == 1:
                    spx = hp // 2
                    src3 = ctxT2[spx].rearrange("p (i n) -> p i n", n=NQ)
                    for qt in range(4):
                        wops = psa.tile([P, D], F32, tag="ps", bufs=3,
                                        name="wo_ps")
                        nc.tensor.matmul(wops[:],
                                         src3[:, :, qt * P:(qt + 1) * P],
                                         wo_sb[:, 2 * spx:2 * spx + 2, :],
                                         start=True, stop=True, perf_mode=DR)
                        if hp == 1:
                            nc.vector.scalar_tensor_tensor(
                                h1acc[qt][:], wops[:], 1.0 / (WS * CS),
                                hq_sb[:, qt:qt + 1, :], op0=ALU.mult, op1=ALU.add)
                        else:
                            nc.vector.scalar_tensor_tensor(
                                h1acc[qt][:], wops[:], 1.0 / (WS * CS),
                                h1acc[qt][:], op0=ALU.mult, op1=ALU.add)

        if STOP_AT == 2:
            for qt in range(4):
                cv = h1pool.tile([P, D], F32, tag="dbg", bufs=2, name="cv")
                nc.vector.tensor_copy(
                    cv[:], ctxT2[qt // 2][:, (qt % 2) * NQ:(qt % 2 + 1) * NQ])
                nc.sync.dma_start(out[qt * P:(qt + 1) * P, :], cv[:])
            qkv_es.close()
            return
        if STOP_AT == 3:
            for qt in range(4):
                nc.sync.dma_start(out[qt * P:(qt + 1) * P, :], h1acc[qt][:])
            qkv_es.close()
            return

        # ---- LN1 (batched stats) + f32 transpose ----
        with tc.tile_pool(name="pspost", bufs=2, space="PSUM") as psw:
            s1 = h1pool.tile([P, 4], F32, tag="s1a", name="s1a")
            s2 = h1pool.tile([P, 4], F32, tag="s2a", name="s2a")
            for qt in range(4):
                nc.vector.reduce_sum(s1[:, qt:qt + 1], h1acc[qt][:],
                                     axis=mybir.AxisListType.X)
                xsq = h1pool.tile([P, D], F32, tag="xsq", bufs=2, name="xsq")
                nc.vector.tensor_mul(xsq[:], h1acc[qt][:], h1acc[qt][:])
                nc.vector.reduce_sum(s2[:, qt:qt + 1], xsq[:],
                                     axis=mybir.AxisListType.X)
            rstd4, nmr4 = _stats4(nc, h1pool, s1, s2, eps_sb, "a")
            for qt in range(4):
                xn = h1pool.tile([P, D], F32, tag="xn", bufs=2, name="xn")
                nc.scalar.activation(xn[:], h1acc[qt][:], AF.Identity,
                                     bias=nmr4[:, qt:qt + 1],
                                     scale=rstd4[:, qt:qt + 1])
                nc.vector.tensor_mul(h1_sb[qt][:], xn[:], g1l)
                nc.vector.tensor_add(h1_sb[qt][:], h1_sb[qt][:], b1l)
                for i in range(4):
                    tp = psw.tile([P, P], F32, tag="tp", name="tp")
                    nc.tensor.transpose(tp[:], h1_sb[qt][:, i * P:(i + 1) * P],
                                        identf_sb[:])
                    nc.vector.tensor_copy(
                        h1T_sb[i][:, qt * P:(qt + 1) * P], tp[:])

        qkv_es.close()

        # ================= FFN (ft-interleaved) =================
        ffnp = es.enter_context(tc.tile_pool(name="ffnp", bufs=1))
        w1_sb = ffnp.tile([P, 4, DFF], BF, tag="w1")
        nc.sync.dma_start(w1_sb[:], w1P[:])
        w2_sb = ffnp.tile([P, 16, D], BF, tag="w2")
        nc.sync.dma_start(w2_sb[:], w2P[:])
        with tc.tile_pool(name="psffn", bufs=1, space="PSUM") as psf:
            ff_ps = [psf.tile([P, D], F32, tag=f"fa{i}", name=f"ff_ps{i}")
                     for i in range(4)]
            for ft in range(16):
                ps = psf.tile([P, NQ], F32, tag="pf", bufs=2, name="f_ps")
                for s in range(4):
                    nc.tensor.matmul(ps[:],
                                     w1_sb[:, s:s + 1, ft * P:(ft + 1) * P],
                                     h1T_sb[s][:], start=(s == 0),
                                     stop=(s == 3))
                fT = ffnp.tile([P, NQ], BF, tag="fT", bufs=3, name="fT")
                nc.scalar.activation(fT[:], ps[:], AF.Relu,
                                     bias=b1_sb[:, ft:ft + 1])
                for qt in range(4):
                    nc.tensor.matmul(ff_ps[qt][:],
                                     fT[:, qt * P:(qt + 1) * P],
                                     w2_sb[:, ft:ft + 1, :], start=(ft == 0),
                                     stop=False)
            # ---- +b2, then LN2 with batched stats ----
            s1 = h1pool.tile([P, 4], F32, tag="s1b", name="s1b")
            s2 = h1pool.tile([P, 4], F32, tag="s2b", name="s2b")
            x2 = []
            for qt in range(4):
                nc.tensor.matmul(ff_ps[qt][:], ones_sb[:], b2_sb[:],
                                 start=False, stop=True)
                x = h1pool.tile([P, D], F32, tag=f"x2{qt}", name=f"x2{qt}")
                nc.vector.scalar_tensor_tensor(x[:], ff_ps[qt][:], 0.0,
                                               h1_sb[qt][:], op0=ALU.add,
                                               op1=ALU.add,
                                               accum_out=s1[:, qt:qt + 1])
                xsq = h1pool.tile([P, D], F32, tag="xsq", bufs=2, name="xsq")
                nc.vector.tensor_mul(xsq[:], x[:], x[:])
                nc.vector.reduce_sum(s2[:, qt:qt + 1], xsq[:],
                                     axis=mybir.AxisListType.X)
                x2.append(x)
            rstd4, nmr4 = _stats4(nc, h1pool, s1, s2, eps_sb, "b")
            for qt in range(4):
                xn = h1pool.tile([P, D], F32, tag="xn", bufs=2, name="xn")
                nc.scalar.activation(xn[:], x2[qt][:], AF.Identity,
                                     bias=nmr4[:, qt:qt + 1],
                                     scale=rstd4[:, qt:qt + 1])
                h2 = h1pool.tile([P, D], F32, tag="h2o", bufs=2, name="h2")
                nc.vector.tensor_mul(h2[:], xn[:], g2l)
                nc.vector.tensor_add(h2[:], h2[:], b2l)
                nc.sync.dma_start(out[qt * P:(qt + 1) * P, :], h2[:])


def _stats4(nc, pool, s1, s2, eps_sb, uid):
    """Batched LN stats: from per-qt sums s1,s2 [P,4] compute rstd4 and
    nmr4 = (-mean * rstd) [P,4]."""
    I32 = mybir.dt.int32
    nm = pool.tile([P, 4], F32, tag="nm4", bufs=2, name=f"nm4{uid}")
    nc.vector.tensor_scalar_mul(nm[:], s1[:], -1.0 / D)
    m2 = pool.tile([P, 4], F32, tag="m24", bufs=2, name=f"m24{uid}")
    nc.vector.tensor_mul(m2[:], nm[:], nm[:])
    var = pool.tile([P, 4], F32, tag="var4", bufs=2, name=f"var4{uid}")
    nc.vector.scalar_tensor_tensor(var[:], s2[:], 1.0 / D, m2[:],
                                   op0=ALU.mult, op1=ALU.subtract)
    ve = pool.tile([P, 4], F32, tag="ve4", bufs=2, name=f"ve4{uid}")
    nc.vector.tensor_scalar_add(ve[:], var[:], eps_sb[:])
    rstd = pool.tile([P, 4], F32, tag="rs4", bufs=2, name=f"rs4{uid}")
    nc.vector.tensor_single_scalar(rstd[:].bitcast(I32), ve[:].bitcast(I32),
                                   1, op=ALU.arith_shift_right)
    nc.vector.tensor_single_scalar(rstd[:].bitcast(I32), rstd[:].bitcast(I32),
                                   0x5F3759DF, op=ALU.subtract)
    nc.vector.tensor_single_scalar(rstd[:].bitcast(I32), rstd[:].bitcast(I32),
                                   -1, op=ALU.mult)
    tq = pool.tile([P, 4], F32, tag="tq4", bufs=2, name=f"tq4{uid}")
    for _ in range(3):
        nc.vector.tensor_mul(tq[:], rstd[:], rstd[:])
        nc.vector.tensor_mul(tq[:], tq[:], ve[:])
        nc.vector.tensor_scalar_mul(tq[:], tq[:], -0.5)
        nc.vector.tensor_scalar_add(tq[:], tq[:], 1.5)
        nc.vector.tensor_mul(rstd[:], rstd[:], tq[:])
    nmr = pool.tile([P, 4], F32, tag="nmr4", bufs=2, name=f"nmr4{uid}")
    nc.vector.tensor_mul(nmr[:], nm[:], rstd[:])
    return rstd, nmr


def _fold(xT):
    """[512, C] -> [128, 4, C] with d = slot*128 + p."""
    c = xT.shape[1]
    return np.ascontiguousarray(xT.reshape(4, P, c).transpose(1, 0, 2))


def _prep_inputs(inputs):
    f32 = np.float32
    h = np.asarray(inputs["h"], f32)
    adj = np.asarray(inputs["adj"])

    def bf(x):
        return np.ascontiguousarray(np.asarray(x, f32).astype(BF16))

    def f8(x):
        return np.ascontiguousarray(np.asarray(x, f32).astype(FP8))

    hT = np.ascontiguousarray(h.T)
    adjb = (adj != 0)
    np.fill_diagonal(adjb, True)
    adjb_bf = adjb.astype(BF16)

    wq, wk, wv, wo = (np.asarray(inputs[k], f32)
                      for k in ("Wq", "Wk", "Wv", "Wo"))
    w1, w2 = np.asarray(inputs["W1"], f32), np.asarray(inputs["W2"], f32)
    bv = np.asarray(inputs["bv"], f32)
    bo = np.asarray(inputs["bo"], f32)
    bo2 = bo + bv @ wo.T  # bv folded through Wo

    lnc = np.concatenate([
        np.broadcast_to(np.asarray(inputs[k], f32), (P, D))
        for k in ("ln1_g", "ln1_b", "ln2_g", "ln2_b")], axis=1)

    shared = {
        "hT2": f8(_fold(hT)),
        "wq2": f8(_fold(wq.T) * WS), "wk2": f8(_fold(wk.T) * WS),
        "wv2": f8(_fold(wv.T) * WS), "wo2": f8(_fold(wo.T) * WS),
        "w1P": bf(w1.T.reshape(4, P, DFF).transpose(1, 0, 2)),
        "w2P": bf(w2.T.reshape(16, P, D).transpose(1, 0, 2)),
        "bqP": np.ascontiguousarray(
            np.asarray(inputs["bq"], f32).reshape(4, P).T),
        "b1P": np.ascontiguousarray(
            np.asarray(inputs["b1"], f32).reshape(16, P).T),
        "b22": bf(np.asarray(inputs["b2"], f32)[None, :]),
        "lnc": np.ascontiguousarray(lnc),
        "identf": np.eye(P, dtype=f32),
    }
    in_maps = []
    for i in range(NCORES):
        r0 = i * NQ
        m = dict(shared)
        m["hqT2"] = f8(_fold(np.ascontiguousarray(hT[:, r0:r0 + NQ])))
        m["hqP"] = np.ascontiguousarray(
            (h[r0:r0 + NQ, :] + bo2).reshape(4, P, D).transpose(1, 0, 2))
        m["maskP"] = np.ascontiguousarray(
            adjb_bf[r0:r0 + NQ, :].T.reshape(32, P, NQ).transpose(1, 0, 2))
        in_maps.append(m)
    return in_maps


def kernel(**inputs) -> np.ndarray:
    global _CACHED, LAST_EXEC_NS
    if _CACHED is None:
        _CACHED = _build()
    nc = _CACHED
    in_maps = _prep_inputs(inputs)
    kw = {}
    if TRACE:
        kw = dict(trace=True, tmpdir=TRACE_DIR)
    res = run_bass_kernel_spmd(nc, in_maps, list(range(NCORES)), **kw)
    LAST_EXEC_NS = res.exec_time_ns
    return np.concatenate([res.results[i]["out"] for i in range(NCORES)],
                          axis=0)


# revision 17
# speedup vs baseline: 1.1730x; 1.0133x over previous
"""GraphTransformerLayer on 8 Trainium2 NeuronCores (Bass/Tile).

Sharding: 8-way along the query-node axis. Each core owns NQ=512 query rows,
computes full K/V projections (replicated), its slice of masked attention,
and its slice of the FFN. No collectives; the host concatenates the slices.

v3:
- Q/K/V and Wo matmuls in fp8e4 DoubleRow perf mode (2 k-subtiles per
  instruction, 0.5 cycles/row); weights host-folded to [128, 4, .], x32
  scaled (ctx x64) for fp8 range. bk dropped (exact under softmax), bv
  folded into host bo' = bo + bv @ Wo.T.
- Wo streams per head-pair into an SBUF f32 accumulator; ctx/wo PSUM tiles
  share one 4-deep ring so normalization lag never stalls the next pair.
- All DMA'd tensors host-packed to their exact SBUF layouts (contiguous,
  hardware-DGE friendly); issued in first-use order, w1/w2 before attention.
- LayerNorms use batched [128,4] stats + rstd across the 4 query tiles,
  ACT-engine center/scale, f32 PE transposes (no bf16 staging copy).
- FFN1/FFN2 interleaved per ft tile; per-qt LN2 + output DMA.
"""

import sys

if "/opt/trn_rl_repo" not in sys.path:
    sys.path.insert(0, "/opt/trn_rl_repo")

import numpy as np
import ml_dtypes

import concourse.bacc as bacc
import concourse.tile as tile
import concourse.mybir as mybir
from concourse.bass_utils import run_bass_kernel_spmd

BF16 = ml_dtypes.bfloat16
FP8 = ml_dtypes.float8_e4m3
F32 = mybir.dt.float32
BF = mybir.dt.bfloat16
F8 = mybir.dt.float8e4

N = 4096
D = 512
H = 8
DK = 64
DFF = 2048
NCORES = 8
NQ = N // NCORES
P = 128
EPS = 1e-5
WS = 32.0  # host weight pre-scale for fp8
CS = 64.0  # ctx pre-scale for fp8

ALU = mybir.AluOpType
AF = mybir.ActivationFunctionType
DR = mybir.MatmulPerfMode.DoubleRow

# set by test.py to capture a profile
TRACE = False
TRACE_DIR = None
LAST_EXEC_NS = None

# debug: truncate after a phase (1=proj, 2=attention ctx, 3=h1acc, 4=full)
STOP_AT = 4

_CACHED = None


def _build():
    nc = bacc.Bacc("TRN2", target_bir_lowering=False, debug=False,
                   num_devices=NCORES)

    # folded fp8 tensors: [128, 4, C]; d = slot*128 + p
    hT2 = nc.dram_tensor("hT2", [P, 4, N], F8, kind="ExternalInput").ap()
    hqT2 = nc.dram_tensor("hqT2", [P, 4, NQ], F8, kind="ExternalInput").ap()
    wq2 = nc.dram_tensor("wq2", [P, 4, D], F8, kind="ExternalInput").ap()
    wk2 = nc.dram_tensor("wk2", [P, 4, D], F8, kind="ExternalInput").ap()
    wv2 = nc.dram_tensor("wv2", [P, 4, D], F8, kind="ExternalInput").ap()
    wo2 = nc.dram_tensor("wo2", [P, 4, D], F8, kind="ExternalInput").ap()
    maskP = nc.dram_tensor("maskP", [P, 32, NQ], BF, kind="ExternalInput").ap()
    w1P = nc.dram_tensor("w1P", [P, 4, DFF], BF, kind="ExternalInput").ap()
    w2P = nc.dram_tensor("w2P", [P, 16, D], BF, kind="ExternalInput").ap()
    bqP = nc.dram_tensor("bqP", [P, 4], F32, kind="ExternalInput").ap()
    b1P = nc.dram_tensor("b1P", [P, 16], F32, kind="ExternalInput").ap()
    b22 = nc.dram_tensor("b22", [1, D], BF, kind="ExternalInput").ap()
    lnc = nc.dram_tensor("lnc", [P, 4 * D], F32, kind="ExternalInput").ap()
    hqP = nc.dram_tensor("hqP", [P, 4, D], F32, kind="ExternalInput").ap()
    identf = nc.dram_tensor("identf", [P, P], F32, kind="ExternalInput").ap()
    out = nc.dram_tensor("out", [NQ, D], F32, kind="ExternalOutput").ap()

    with tile.TileContext(nc) as tc:
        _emit(nc, tc, locals())
    nc.compile()
    return nc


def _emit(nc, tc, t):
    hT2, hqT2, maskP = t["hT2"], t["hqT2"], t["maskP"]
    wq2, wk2, wv2, wo2 = t["wq2"], t["wk2"], t["wv2"], t["wo2"]
    w1P, w2P = t["w1P"], t["w2P"]
    bqP, b1P, b22, lnc, hqP = t["bqP"], t["b1P"], t["b22"], t["lnc"], t["hqP"]
    identf, out = t["identf"], t["out"]

    from contextlib import ExitStack

    es = ExitStack()
    with es:
        cpool = es.enter_context(tc.tile_pool(name="const", bufs=1))
        h1pool = es.enter_context(tc.tile_pool(name="h1p", bufs=1))
        qkv_es = ExitStack()
        qkvpool = qkv_es.enter_context(tc.tile_pool(name="qkvp", bufs=1))
        mpool = qkv_es.enter_context(tc.tile_pool(name="maskp", bufs=1))
        proj_es = ExitStack()
        ppool = proj_es.enter_context(tc.tile_pool(name="projp", bufs=1))

        # ---- DMAs, first-use order, all contiguous host-packed ----
        wq_sb = ppool.tile([P, 4, D], F8, tag="wq")
        nc.sync.dma_start(wq_sb[:], wq2[:])
        hqT_sb = ppool.tile([P, 4, NQ], F8, tag="hqT")
        nc.sync.dma_start(hqT_sb[:], hqT2[:])
        wk_sb = ppool.tile([P, 4, D], F8, tag="wk")
        nc.sync.dma_start(wk_sb[:], wk2[:])
        hT_sb = ppool.tile([P, 4, N], F8, tag="hT")
        nc.sync.dma_start(hT_sb[:], hT2[:])
        wv_sb = ppool.tile([P, 4, D], F8, tag="wv")
        nc.sync.dma_start(wv_sb[:], wv2[:])
        bq_sb = cpool.tile([P, 4], F32, tag="bq")
        nc.sync.dma_start(bq_sb[:], bqP[:])
        # mask in SBUF layout [128, mt, NQ], two halves on the gpsimd queue
        mask_sb = mpool.tile([P, 32, NQ], BF, tag="mask")
        nc.gpsimd.dma_start(mask_sb[:, 0:16, :], maskP[:, 0:16, :])
        nc.gpsimd.dma_start(mask_sb[:, 16:32, :], maskP[:, 16:32, :])
        wo_sb = cpool.tile([P, 4, D], F8, tag="wo")
        nc.sync.dma_start(wo_sb[:], wo2[:])
        lnab = cpool.tile([P, 4 * D], F32, tag="lnc")
        nc.sync.dma_start(lnab[:], lnc[:])
        hq_sb = cpool.tile([P, 4, D], F32, tag="hq")
        nc.sync.dma_start(hq_sb[:], hqP[:])
        b1_sb = cpool.tile([P, 16], F32, tag="b1")
        nc.sync.dma_start(b1_sb[:], b1P[:])
        b2_sb = cpool.tile([1, D], BF, tag="b2")
        nc.sync.dma_start(b2_sb[:], b22[:])
        identf_sb = cpool.tile([P, P], F32, tag="idf")
        nc.sync.dma_start(identf_sb[:], identf[:])
        ones_sb = cpool.tile([1, P], BF, tag="ones")
        nc.vector.memset(ones_sb[:], 1.0)
        eps_sb = cpool.tile([P, 1], F32, tag="eps")
        nc.vector.memset(eps_sb[:], EPS)

        g1l = lnab[:, 0:D]
        b1l = lnab[:, D:2 * D]
        g2l = lnab[:, 2 * D:3 * D]
        b2l = lnab[:, 3 * D:4 * D]

        # ---- persistent attention state ----
        kT_sb = [qkvpool.tile([P, N], BF, tag=f"kt{i}", name=f"kT{i}")
                 for i in range(4)]
        qT_sb = [qkvpool.tile([P, NQ], BF, tag=f"qt{i}", name=f"qT{i}")
                 for i in range(4)]
        v_sb = [qkvpool.tile([P, H * (DK + 1)], BF, tag=f"v{i}", name=f"v{i}")
                for i in range(32)]
        for mt in range(32):
            vv = v_sb[mt].rearrange("p (h c) -> p h c", c=DK + 1)
            nc.vector.memset(vv[:, :, DK:DK + 1], 1.0)
        ctxT2 = [h1pool.tile([P, 2 * NQ], F8, tag=f"cx{i}", name=f"ctxT2{i}")
                 for i in range(2)]
        h1acc = [h1pool.tile([P, D], F32, tag=f"ha{i}", name=f"h1acc{i}")
                 for i in range(4)]
        h1_sb = [h1pool.tile([P, D], F32, tag=f"h1_{i}", name=f"h1_{i}")
                 for i in range(4)]
        h1T_sb = [h1pool.tile([P, NQ], BF, tag=f"h1T{i}", name=f"h1T{i}")
                  for i in range(4)]

        # ================= projections (fp8 DoubleRow) =================
        with tc.tile_pool(name="psproj", bufs=4, space="PSUM") as psp:
            for tt in range(4):
                ps = psp.tile([P, NQ], F32, tag="pp", name="ps_q")
                for sp in range(2):
                    nc.tensor.matmul(ps[:],
                                     wq_sb[:, 2 * sp:2 * sp + 2,
                                           tt * P:(tt + 1) * P],
                                     hqT_sb[:, 2 * sp:2 * sp + 2, :],
                                     start=(sp == 0), stop=(sp == 1),
                                     perf_mode=DR)
                nc.scalar.activation(qT_sb[tt][:], ps[:], AF.Identity,
                                     bias=bq_sb[:, tt:tt + 1], scale=1.0 / WS)
            for tt in range(4):
                for c in range(8):
                    ps = psp.tile([P, D], F32, tag="pp", name="ps_k")
                    for sp in range(2):
                        nc.tensor.matmul(ps[:],
                                         wk_sb[:, 2 * sp:2 * sp + 2,
                                               tt * P:(tt + 1) * P],
                                         hT_sb[:, 2 * sp:2 * sp + 2,
                                               c * D:(c + 1) * D],
                                         start=(sp == 0), stop=(sp == 1),
                                         perf_mode=DR)
                    nc.vector.tensor_scalar_mul(
                        kT_sb[tt][:, c * D:(c + 1) * D], ps[:], 1.0 / WS)
            for mc in range(32):
                ps = psp.tile([P, D], F32, tag="pp", name="ps_v")
                for sp in range(2):
                    nc.tensor.matmul(ps[:],
                                     hT_sb[:, 2 * sp:2 * sp + 2,
                                           mc * P:(mc + 1) * P],
                                     wv_sb[:, 2 * sp:2 * sp + 2, :],
                                     start=(sp == 0), stop=(sp == 1),
                                     perf_mode=DR)
                vv = v_sb[mc].rearrange("p (h c) -> p h c", c=DK + 1)
                nc.scalar.activation(vv[:, :, 0:DK],
                                     ps.rearrange("p (h c) -> p h c", c=DK),
                                     AF.Copy, scale=1.0 / WS)
        proj_es.close()

        if STOP_AT == 1:
            for qt in range(4):
                cv = h1pool.tile([P, D], F32, tag="dbg", bufs=2, name="cv")
                nc.vector.tensor_add(cv[:], kT_sb[qt][:, 0:D], qT_sb[qt][:])
                nc.vector.tensor_add(cv[:], cv[:], v_sb[qt * 8][:, 0:D])
                nc.sync.dma_start(out[qt * P:(qt + 1) * P, :], cv[:])
            qkv_es.close()
            return

        # ================= attention =================
        with tc.tile_pool(name="attp", bufs=1) as apool, \
             tc.tile_pool(name="psatt", bufs=1, space="PSUM") as psa:
            for hp in range(4):
                ctx_ps = psa.tile([P, 2 * NQ], F32, tag="pc", bufs=1,
                                  name="ctx_ps")
                for g in range(16):
                    sp = [psa.tile([P, 1024], F32, tag="ps", bufs=3,
                                   name="sc_ps") for _ in range(2)]
                    at = [apool.tile([P, 1024], BF, tag="at", bufs=6,
                                     name="at") for _ in range(2)]
                    for i, po in ((0, 0), (1, DK)):
                        for j in range(2):
                            mt = 2 * g + j
                            nc.tensor.matmul(
                                sp[i][:, j * NQ:(j + 1) * NQ],
                                kT_sb[hp][po:po + DK, mt * P:(mt + 1) * P],
                                qT_sb[hp][po:po + DK, :],
                                start=True, stop=True)
                    for i in range(2):
                        nc.scalar.activation(at[i][:], sp[i][:], AF.Exp,
                                             scale=0.125)
                        nc.vector.tensor_mul(
                            at[i][:], at[i][:],
                            mask_sb[:, 2 * g:2 * g + 2, :])
                    for i, h in ((0, 2 * hp), (1, 2 * hp + 1)):
                        for j in range(2):
                            mt = 2 * g + j
                            nc.tensor.matmul(
                                ctx_ps[0:DK + 1, i * NQ:(i + 1) * NQ],
                                v_sb[mt][:, h * 65:h * 65 + 65],
                                at[i][:, j * NQ:(j + 1) * NQ],
                                start=(mt == 0), stop=(mt == 31))
                # evict ctx+rowsums to SBUF fast (frees the psum bank for
                # the next head-pair), then normalize from SBUF
                ctxe = apool.tile([DK + 1, 2 * NQ], F32, tag="ce", bufs=2,
                                  name="ctxe")
                nc.vector.tensor_copy(ctxe[:], ctx_ps[0:DK + 1, :])
                dst = ctxT2[hp // 2]
                col = (hp % 2) * NQ
                for i, po in ((0, 0), (1, DK)):
                    rec = apool.tile([1, NQ], F32, tag="rec", bufs=2,
                                     name="rec")
                    nc.vector.reciprocal(
                        rec[:], ctxe[DK:DK + 1, i * NQ:(i + 1) * NQ])
                    bc = apool.tile([P, NQ], F32, tag="bc", bufs=2, name="bc")
                    nc.gpsimd.partition_broadcast(bc[:], rec[:])
                    nc.vector.scalar_tensor_tensor(
                        dst[po:po + DK, col:col + NQ],
                        ctxe[0:DK, i * NQ:(i + 1) * NQ],
                        CS, bc[0:DK, :], op0=ALU.mult, op1=ALU.mult)
                if STOP_AT == 2:
                    continue
                # stream Wo for the completed head-pair (fp8 DoubleRow)
                if hp % 2 == 1:
                    spx = hp // 2
                    src3 = ctxT2[spx].rearrange("p (i n) -> p i n", n=NQ)
                    for qt in range(4):
                        wops = psa.tile([P, D], F32, tag="ps", bufs=3,
                                        name="wo_ps")
                        nc.tensor.matmul(wops[:],
                                         src3[:, :, qt * P:(qt + 1) * P],
                                         wo_sb[:, 2 * spx:2 * spx + 2, :],
                                         start=True, stop=True, perf_mode=DR)
                        if hp == 1:
                            nc.vector.scalar_tensor_tensor(
                                h1acc[qt][:], wops[:], 1.0 / (WS * CS),
                                hq_sb[:, qt:qt + 1, :], op0=ALU.mult, op1=ALU.add)
                        else:
                            nc.vector.scalar_tensor_tensor(
                                h1acc[qt][:], wops[:], 1.0 / (WS * CS),
                                h1acc[qt][:], op0=ALU.mult, op1=ALU.add)

        if STOP_AT == 2:
            for qt in range(4):
                cv = h1pool.tile([P, D], F32, tag="dbg", bufs=2, name="cv")
                nc.vector.tensor_copy(
                    cv[:], ctxT2[qt // 2][:, (qt % 2) * NQ:(qt % 2 + 1) * NQ])
                nc.sync.dma_start(out[qt * P:(qt + 1) * P, :], cv[:])
            qkv_es.close()
            return
        if STOP_AT == 3:
            for qt in range(4):
                nc.sync.dma_start(out[qt * P:(qt + 1) * P, :], h1acc[qt][:])
            qkv_es.close()
            return

        # ---- LN1 (batched stats) + f32 transpose ----
        with tc.tile_pool(name="pspost", bufs=2, space="PSUM") as psw:
            s1 = h1pool.tile([P, 4], F32, tag="s1a", name="s1a")
            s2 = h1pool.tile([P, 4], F32, tag="s2a", name="s2a")
            for qt in range(4):
                nc.vector.reduce_sum(s1[:, qt:qt + 1], h1acc[qt][:],
                                     axis=mybir.AxisListType.X)
                xsq = h1pool.tile([P, D], F32, tag="xsq", bufs=4, name="xsq")
                eng = nc.vector if qt < 2 else nc.gpsimd
                eng.tensor_mul(xsq[:], h1acc[qt][:], h1acc[qt][:])
                nc.vector.reduce_sum(s2[:, qt:qt + 1], xsq[:],
                                     axis=mybir.AxisListType.X)
            rstd4, nmr4 = _stats4(nc, h1pool, s1, s2, eps_sb, "a")
            for qt in range(4):
                xn = h1pool.tile([P, D], F32, tag="xn", bufs=2, name="xn")
                nc.scalar.activation(xn[:], h1acc[qt][:], AF.Identity,
                                     bias=nmr4[:, qt:qt + 1],
                                     scale=rstd4[:, qt:qt + 1])
                nc.vector.tensor_mul(h1_sb[qt][:], xn[:], g1l)
                nc.vector.tensor_add(h1_sb[qt][:], h1_sb[qt][:], b1l)
                for i in range(4):
                    tp = psw.tile([P, P], F32, tag="tp", name="tp")
                    nc.tensor.transpose(tp[:], h1_sb[qt][:, i * P:(i + 1) * P],
                                        identf_sb[:])
                    nc.vector.tensor_copy(
                        h1T_sb[i][:, qt * P:(qt + 1) * P], tp[:])

        qkv_es.close()

        # ================= FFN (ft-interleaved) =================
        ffnp = es.enter_context(tc.tile_pool(name="ffnp", bufs=1))
        w1_sb = ffnp.tile([P, 4, DFF], BF, tag="w1")
        nc.sync.dma_start(w1_sb[:], w1P[:])
        w2_sb = ffnp.tile([P, 16, D], BF, tag="w2")
        nc.sync.dma_start(w2_sb[:], w2P[:])
        with tc.tile_pool(name="psffn", bufs=1, space="PSUM") as psf:
            ff_ps = [psf.tile([P, D], F32, tag=f"fa{i}", name=f"ff_ps{i}")
                     for i in range(4)]
            for ft in range(16):
                ps = psf.tile([P, NQ], F32, tag="pf", bufs=2, name="f_ps")
                for s in range(4):
                    nc.tensor.matmul(ps[:],
                                     w1_sb[:, s:s + 1, ft * P:(ft + 1) * P],
                                     h1T_sb[s][:], start=(s == 0),
                                     stop=(s == 3))
                fT = ffnp.tile([P, NQ], BF, tag="fT", bufs=3, name="fT")
                nc.scalar.activation(fT[:], ps[:], AF.Relu,
                                     bias=b1_sb[:, ft:ft + 1])
                for qt in range(4):
                    nc.tensor.matmul(ff_ps[qt][:],
                                     fT[:, qt * P:(qt + 1) * P],
                                     w2_sb[:, ft:ft + 1, :], start=(ft == 0),
                                     stop=False)
            # ---- +b2, then LN2 with batched stats ----
            s1 = h1pool.tile([P, 4], F32, tag="s1b", name="s1b")
            s2 = h1pool.tile([P, 4], F32, tag="s2b", name="s2b")
            x2 = []
            for qt in range(4):
                nc.tensor.matmul(ff_ps[qt][:], ones_sb[:], b2_sb[:],
                                 start=False, stop=True)
                x = h1pool.tile([P, D], F32, tag=f"x2{qt}", name=f"x2{qt}")
                nc.vector.scalar_tensor_tensor(x[:], ff_ps[qt][:], 0.0,
                                               h1_sb[qt][:], op0=ALU.add,
                                               op1=ALU.add,
                                               accum_out=s1[:, qt:qt + 1])
                xsq = h1pool.tile([P, D], F32, tag="xsq", bufs=4, name="xsq")
                eng = nc.vector if qt < 2 else nc.gpsimd
                eng.tensor_mul(xsq[:], x[:], x[:])
                nc.vector.reduce_sum(s2[:, qt:qt + 1], xsq[:],
                                     axis=mybir.AxisListType.X)
                x2.append(x)
            rstd4, nmr4 = _stats4(nc, h1pool, s1, s2, eps_sb, "b")
            for qt in range(4):
                xn = h1pool.tile([P, D], F32, tag="xn", bufs=2, name="xn")
                nc.scalar.activation(xn[:], x2[qt][:], AF.Identity,
                                     bias=nmr4[:, qt:qt + 1],
                                     scale=rstd4[:, qt:qt + 1])
                h2 = h1pool.tile([P, D], F32, tag="h2o", bufs=2, name="h2")
                nc.vector.tensor_mul(h2[:], xn[:], g2l)
                nc.vector.tensor_add(h2[:], h2[:], b2l)
                nc.sync.dma_start(out[qt * P:(qt + 1) * P, :], h2[:])


def _stats4(nc, pool, s1, s2, eps_sb, uid):
    """Batched LN stats: from per-qt sums s1,s2 [P,4] compute rstd4 and
    nmr4 = (-mean * rstd) [P,4]."""
    I32 = mybir.dt.int32
    nm = pool.tile([P, 4], F32, tag="nm4", bufs=2, name=f"nm4{uid}")
    nc.vector.tensor_scalar_mul(nm[:], s1[:], -1.0 / D)
    m2 = pool.tile([P, 4], F32, tag="m24", bufs=2, name=f"m24{uid}")
    nc.vector.tensor_mul(m2[:], nm[:], nm[:])
    var = pool.tile([P, 4], F32, tag="var4", bufs=2, name=f"var4{uid}")
    nc.vector.scalar_tensor_tensor(var[:], s2[:], 1.0 / D, m2[:],
                                   op0=ALU.mult, op1=ALU.subtract)
    ve = pool.tile([P, 4], F32, tag="ve4", bufs=2, name=f"ve4{uid}")
    nc.vector.tensor_scalar_add(ve[:], var[:], eps_sb[:])
    rstd = pool.tile([P, 4], F32, tag="rs4", bufs=2, name=f"rs4{uid}")
    nc.vector.tensor_single_scalar(rstd[:].bitcast(I32), ve[:].bitcast(I32),
                                   1, op=ALU.arith_shift_right)
    nc.vector.tensor_single_scalar(rstd[:].bitcast(I32), rstd[:].bitcast(I32),
                                   0x5F3759DF, op=ALU.subtract)
    nc.vector.tensor_single_scalar(rstd[:].bitcast(I32), rstd[:].bitcast(I32),
                                   -1, op=ALU.mult)
    tq = pool.tile([P, 4], F32, tag="tq4", bufs=2, name=f"tq4{uid}")
    for _ in range(3):
        nc.vector.tensor_mul(tq[:], rstd[:], rstd[:])
        nc.vector.tensor_mul(tq[:], tq[:], ve[:])
        nc.vector.tensor_scalar_mul(tq[:], tq[:], -0.5)
        nc.vector.tensor_scalar_add(tq[:], tq[:], 1.5)
        nc.vector.tensor_mul(rstd[:], rstd[:], tq[:])
    nmr = pool.tile([P, 4], F32, tag="nmr4", bufs=2, name=f"nmr4{uid}")
    nc.vector.tensor_mul(nmr[:], nm[:], rstd[:])
    return rstd, nmr


def _fold(xT):
    """[512, C] -> [128, 4, C] with d = slot*128 + p."""
    c = xT.shape[1]
    return np.ascontiguousarray(xT.reshape(4, P, c).transpose(1, 0, 2))


def _prep_inputs(inputs):
    f32 = np.float32
    h = np.asarray(inputs["h"], f32)
    adj = np.asarray(inputs["adj"])

    def bf(x):
        return np.ascontiguousarray(np.asarray(x, f32).astype(BF16))

    def f8(x):
        return np.ascontiguousarray(np.asarray(x, f32).astype(FP8))

    hT = np.ascontiguousarray(h.T)
    adjb = (adj != 0)
    np.fill_diagonal(adjb, True)
    adjb_bf = adjb.astype(BF16)

    wq, wk, wv, wo = (np.asarray(inputs[k], f32)
                      for k in ("Wq", "Wk", "Wv", "Wo"))
    w1, w2 = np.asarray(inputs["W1"], f32), np.asarray(inputs["W2"], f32)
    bv = np.asarray(inputs["bv"], f32)
    bo = np.asarray(inputs["bo"], f32)
    bo2 = bo + bv @ wo.T  # bv folded through Wo

    lnc = np.concatenate([
        np.broadcast_to(np.asarray(inputs[k], f32), (P, D))
        for k in ("ln1_g", "ln1_b", "ln2_g", "ln2_b")], axis=1)

    shared = {
        "hT2": f8(_fold(hT)),
        "wq2": f8(_fold(wq.T) * WS), "wk2": f8(_fold(wk.T) * WS),
        "wv2": f8(_fold(wv.T) * WS), "wo2": f8(_fold(wo.T) * WS),
        "w1P": bf(w1.T.reshape(4, P, DFF).transpose(1, 0, 2)),
        "w2P": bf(w2.T.reshape(16, P, D).transpose(1, 0, 2)),
        "bqP": np.ascontiguousarray(
            np.asarray(inputs["bq"], f32).reshape(4, P).T),
        "b1P": np.ascontiguousarray(
            np.asarray(inputs["b1"], f32).reshape(16, P).T),
        "b22": bf(np.asarray(inputs["b2"], f32)[None, :]),
        "lnc": np.ascontiguousarray(lnc),
        "identf": np.eye(P, dtype=f32),
    }
    in_maps = []
    for i in range(NCORES):
        r0 = i * NQ
        m = dict(shared)
        m["hqT2"] = f8(_fold(np.ascontiguousarray(hT[:, r0:r0 + NQ])))
        m["hqP"] = np.ascontiguousarray(
            (h[r0:r0 + NQ, :] + bo2).reshape(4, P, D).transpose(1, 0, 2))
        m["maskP"] = np.ascontiguousarray(
            adjb_bf[r0:r0 + NQ, :].T.reshape(32, P, NQ).transpose(1, 0, 2))
        in_maps.append(m)
    return in_maps


def kernel(**inputs) -> np.ndarray:
    global _CACHED, LAST_EXEC_NS
    if _CACHED is None:
        _CACHED = _build()
    nc = _CACHED
    in_maps = _prep_inputs(inputs)
    kw = {}
    if TRACE:
        kw = dict(trace=True, tmpdir=TRACE_DIR)
    res = run_bass_kernel_spmd(nc, in_maps, list(range(NCORES)), **kw)
    LAST_EXEC_NS = res.exec_time_ns
    return np.concatenate([res.results[i]["out"] for i in range(NCORES)],
                          axis=0)


# revision 18
# speedup vs baseline: 1.1954x; 1.0191x over previous
"""GraphTransformerLayer on 8 Trainium2 NeuronCores (Bass/Tile).

Sharding: 8-way along the query-node axis. Each core owns NQ=512 query rows,
computes full K/V projections (replicated), its slice of masked attention,
and its slice of the FFN. No collectives; the host concatenates the slices.

v3:
- Q/K/V and Wo matmuls in fp8e4 DoubleRow perf mode (2 k-subtiles per
  instruction, 0.5 cycles/row); weights host-folded to [128, 4, .], x32
  scaled (ctx x64) for fp8 range. bk dropped (exact under softmax), bv
  folded into host bo' = bo + bv @ Wo.T.
- Wo streams per head-pair into an SBUF f32 accumulator; ctx/wo PSUM tiles
  share one 4-deep ring so normalization lag never stalls the next pair.
- All DMA'd tensors host-packed to their exact SBUF layouts (contiguous,
  hardware-DGE friendly); issued in first-use order, w1/w2 before attention.
- LayerNorms use batched [128,4] stats + rstd across the 4 query tiles,
  ACT-engine center/scale, f32 PE transposes (no bf16 staging copy).
- FFN1/FFN2 interleaved per ft tile; per-qt LN2 + output DMA.
"""

import sys

if "/opt/trn_rl_repo" not in sys.path:
    sys.path.insert(0, "/opt/trn_rl_repo")

import numpy as np
import ml_dtypes

import concourse.bacc as bacc
import concourse.tile as tile
import concourse.mybir as mybir
from concourse.bass_utils import run_bass_kernel_spmd

BF16 = ml_dtypes.bfloat16
FP8 = ml_dtypes.float8_e4m3
F32 = mybir.dt.float32
BF = mybir.dt.bfloat16
F8 = mybir.dt.float8e4

N = 4096
D = 512
H = 8
DK = 64
DFF = 2048
NCORES = 8
NQ = N // NCORES
P = 128
EPS = 1e-5
WS = 32.0  # host weight pre-scale for fp8
CS = 64.0  # ctx pre-scale for fp8

ALU = mybir.AluOpType
AF = mybir.ActivationFunctionType
DR = mybir.MatmulPerfMode.DoubleRow

# set by test.py to capture a profile
TRACE = False
TRACE_DIR = None
LAST_EXEC_NS = None

# debug: truncate after a phase (1=proj, 2=attention ctx, 3=h1acc, 4=full)
STOP_AT = 4

_CACHED = None


def _build():
    nc = bacc.Bacc("TRN2", target_bir_lowering=False, debug=False,
                   num_devices=NCORES)

    # folded fp8 tensors: [128, 4, C]; d = slot*128 + p
    hT2 = nc.dram_tensor("hT2", [P, 4, N], F8, kind="ExternalInput").ap()
    hqT2 = nc.dram_tensor("hqT2", [P, 4, NQ], F8, kind="ExternalInput").ap()
    wq2 = nc.dram_tensor("wq2", [P, 4, D], F8, kind="ExternalInput").ap()
    wk2 = nc.dram_tensor("wk2", [P, 4, D], F8, kind="ExternalInput").ap()
    wv2 = nc.dram_tensor("wv2", [P, 4, D], F8, kind="ExternalInput").ap()
    wo2 = nc.dram_tensor("wo2", [P, 4, D], F8, kind="ExternalInput").ap()
    maskP = nc.dram_tensor("maskP", [P, 32, NQ], BF, kind="ExternalInput").ap()
    w1P = nc.dram_tensor("w1P", [P, 4, DFF], BF, kind="ExternalInput").ap()
    w2P = nc.dram_tensor("w2P", [P, 16, D], BF, kind="ExternalInput").ap()
    bqP = nc.dram_tensor("bqP", [P, 4], F32, kind="ExternalInput").ap()
    b1P = nc.dram_tensor("b1P", [P, 16], F32, kind="ExternalInput").ap()
    b22 = nc.dram_tensor("b22", [1, D], BF, kind="ExternalInput").ap()
    lnc = nc.dram_tensor("lnc", [P, 4 * D], F32, kind="ExternalInput").ap()
    hqP = nc.dram_tensor("hqP", [P, 4, D], F32, kind="ExternalInput").ap()
    identf = nc.dram_tensor("identf", [P, P], F32, kind="ExternalInput").ap()
    out = nc.dram_tensor("out", [NQ, D], F32, kind="ExternalOutput").ap()

    with tile.TileContext(nc) as tc:
        _emit(nc, tc, locals())
    nc.compile()
    return nc


def _emit(nc, tc, t):
    hT2, hqT2, maskP = t["hT2"], t["hqT2"], t["maskP"]
    wq2, wk2, wv2, wo2 = t["wq2"], t["wk2"], t["wv2"], t["wo2"]
    w1P, w2P = t["w1P"], t["w2P"]
    bqP, b1P, b22, lnc, hqP = t["bqP"], t["b1P"], t["b22"], t["lnc"], t["hqP"]
    identf, out = t["identf"], t["out"]

    from contextlib import ExitStack

    es = ExitStack()
    with es:
        cpool = es.enter_context(tc.tile_pool(name="const", bufs=1))
        h1pool = es.enter_context(tc.tile_pool(name="h1p", bufs=1))
        qkv_es = ExitStack()
        qkvpool = qkv_es.enter_context(tc.tile_pool(name="qkvp", bufs=1))
        mpool = qkv_es.enter_context(tc.tile_pool(name="maskp", bufs=1))
        proj_es = ExitStack()
        ppool = proj_es.enter_context(tc.tile_pool(name="projp", bufs=1))

        # ---- DMAs, first-use order, all contiguous host-packed ----
        wq_sb = ppool.tile([P, 4, D], F8, tag="wq")
        nc.sync.dma_start(wq_sb[:], wq2[:])
        hqT_sb = ppool.tile([P, 4, NQ], F8, tag="hqT")
        nc.sync.dma_start(hqT_sb[:], hqT2[:])
        bq_sb = cpool.tile([P, 4], F32, tag="bq")
        nc.sync.dma_start(bq_sb[:], bqP[:])
        wk_sb = ppool.tile([P, 4, D], F8, tag="wk")
        nc.sync.dma_start(wk_sb[:], wk2[:])
        hT_sb = ppool.tile([P, 4, N], F8, tag="hT")
        for ck in range(4):
            nc.sync.dma_start(hT_sb[:, :, ck * (N // 4):(ck + 1) * (N // 4)],
                              hT2[:, :, ck * (N // 4):(ck + 1) * (N // 4)])
        wv_sb = ppool.tile([P, 4, D], F8, tag="wv")
        nc.sync.dma_start(wv_sb[:], wv2[:])
        # mask in SBUF layout [128, mt, NQ], two halves on the gpsimd queue
        mask_sb = mpool.tile([P, 32, NQ], BF, tag="mask")
        nc.gpsimd.dma_start(mask_sb[:, 0:16, :], maskP[:, 0:16, :])
        nc.gpsimd.dma_start(mask_sb[:, 16:32, :], maskP[:, 16:32, :])
        wo_sb = cpool.tile([P, 4, D], F8, tag="wo")
        nc.sync.dma_start(wo_sb[:], wo2[:])
        lnab = cpool.tile([P, 4 * D], F32, tag="lnc")
        nc.sync.dma_start(lnab[:], lnc[:])
        hq_sb = cpool.tile([P, 4, D], F32, tag="hq")
        nc.sync.dma_start(hq_sb[:], hqP[:])
        b1_sb = cpool.tile([P, 16], F32, tag="b1")
        nc.sync.dma_start(b1_sb[:], b1P[:])
        b2_sb = cpool.tile([1, D], BF, tag="b2")
        nc.sync.dma_start(b2_sb[:], b22[:])
        identf_sb = cpool.tile([P, P], F32, tag="idf")
        nc.sync.dma_start(identf_sb[:], identf[:])
        ones_sb = cpool.tile([1, P], BF, tag="ones")
        nc.vector.memset(ones_sb[:], 1.0)
        eps_sb = cpool.tile([P, 1], F32, tag="eps")
        nc.vector.memset(eps_sb[:], EPS)

        g1l = lnab[:, 0:D]
        b1l = lnab[:, D:2 * D]
        g2l = lnab[:, 2 * D:3 * D]
        b2l = lnab[:, 3 * D:4 * D]

        # ---- persistent attention state ----
        kT_sb = [qkvpool.tile([P, N], BF, tag=f"kt{i}", name=f"kT{i}")
                 for i in range(4)]
        qT_sb = [qkvpool.tile([P, NQ], BF, tag=f"qt{i}", name=f"qT{i}")
                 for i in range(4)]
        v_sb = [qkvpool.tile([P, H * (DK + 1)], BF, tag=f"v{i}", name=f"v{i}")
                for i in range(32)]
        for mt in range(32):
            vv = v_sb[mt].rearrange("p (h c) -> p h c", c=DK + 1)
            nc.vector.memset(vv[:, :, DK:DK + 1], 1.0)
        ctxT2 = [h1pool.tile([P, 2 * NQ], F8, tag=f"cx{i}", name=f"ctxT2{i}")
                 for i in range(2)]
        h1acc = [h1pool.tile([P, D], F32, tag=f"ha{i}", name=f"h1acc{i}")
                 for i in range(4)]
        h1_sb = [h1pool.tile([P, D], F32, tag=f"h1_{i}", name=f"h1_{i}")
                 for i in range(4)]
        h1T_sb = [h1pool.tile([P, NQ], BF, tag=f"h1T{i}", name=f"h1T{i}")
                  for i in range(4)]

        # ================= projections (fp8 DoubleRow) =================
        with tc.tile_pool(name="psproj", bufs=4, space="PSUM") as psp:
            for tt in range(4):
                ps = psp.tile([P, NQ], F32, tag="pp", name="ps_q")
                for sp in range(2):
                    nc.tensor.matmul(ps[:],
                                     wq_sb[:, 2 * sp:2 * sp + 2,
                                           tt * P:(tt + 1) * P],
                                     hqT_sb[:, 2 * sp:2 * sp + 2, :],
                                     start=(sp == 0), stop=(sp == 1),
                                     perf_mode=DR)
                nc.scalar.activation(qT_sb[tt][:], ps[:], AF.Identity,
                                     bias=bq_sb[:, tt:tt + 1], scale=1.0 / WS)
            for tt in range(4):
                for c in range(8):
                    ps = psp.tile([P, D], F32, tag="pp", name="ps_k")
                    for sp in range(2):
                        nc.tensor.matmul(ps[:],
                                         wk_sb[:, 2 * sp:2 * sp + 2,
                                               tt * P:(tt + 1) * P],
                                         hT_sb[:, 2 * sp:2 * sp + 2,
                                               c * D:(c + 1) * D],
                                         start=(sp == 0), stop=(sp == 1),
                                         perf_mode=DR)
                    nc.vector.tensor_scalar_mul(
                        kT_sb[tt][:, c * D:(c + 1) * D], ps[:], 1.0 / WS)
            for mc in range(32):
                ps = psp.tile([P, D], F32, tag="pp", name="ps_v")
                for sp in range(2):
                    nc.tensor.matmul(ps[:],
                                     hT_sb[:, 2 * sp:2 * sp + 2,
                                           mc * P:(mc + 1) * P],
                                     wv_sb[:, 2 * sp:2 * sp + 2, :],
                                     start=(sp == 0), stop=(sp == 1),
                                     perf_mode=DR)
                vv = v_sb[mc].rearrange("p (h c) -> p h c", c=DK + 1)
                nc.scalar.activation(vv[:, :, 0:DK],
                                     ps.rearrange("p (h c) -> p h c", c=DK),
                                     AF.Copy, scale=1.0 / WS)
        proj_es.close()

        if STOP_AT == 1:
            for qt in range(4):
                cv = h1pool.tile([P, D], F32, tag="dbg", bufs=2, name="cv")
                nc.vector.tensor_add(cv[:], kT_sb[qt][:, 0:D], qT_sb[qt][:])
                nc.vector.tensor_add(cv[:], cv[:], v_sb[qt * 8][:, 0:D])
                nc.sync.dma_start(out[qt * P:(qt + 1) * P, :], cv[:])
            qkv_es.close()
            return

        # ================= attention =================
        with tc.tile_pool(name="attp", bufs=1) as apool, \
             tc.tile_pool(name="psatt", bufs=1, space="PSUM") as psa:
            for hp in range(4):
                ctx_ps = psa.tile([P, 2 * NQ], F32, tag="pc", bufs=1,
                                  name="ctx_ps")
                for g in range(16):
                    sp = [psa.tile([P, 1024], F32, tag="ps", bufs=3,
                                   name="sc_ps") for _ in range(2)]
                    at = [apool.tile([P, 1024], BF, tag="at", bufs=6,
                                     name="at") for _ in range(2)]
                    for i, po in ((0, 0), (1, DK)):
                        for j in range(2):
                            mt = 2 * g + j
                            nc.tensor.matmul(
                                sp[i][:, j * NQ:(j + 1) * NQ],
                                kT_sb[hp][po:po + DK, mt * P:(mt + 1) * P],
                                qT_sb[hp][po:po + DK, :],
                                start=True, stop=True)
                    for i in range(2):
                        nc.scalar.activation(at[i][:], sp[i][:], AF.Exp,
                                             scale=0.125)
                        nc.vector.tensor_mul(
                            at[i][:], at[i][:],
                            mask_sb[:, 2 * g:2 * g + 2, :])
                    for i, h in ((0, 2 * hp), (1, 2 * hp + 1)):
                        for j in range(2):
                            mt = 2 * g + j
                            nc.tensor.matmul(
                                ctx_ps[0:DK + 1, i * NQ:(i + 1) * NQ],
                                v_sb[mt][:, h * 65:h * 65 + 65],
                                at[i][:, j * NQ:(j + 1) * NQ],
                                start=(mt == 0), stop=(mt == 31))
                # evict ctx+rowsums to SBUF fast (frees the psum bank for
                # the next head-pair), then normalize from SBUF
                ctxe = apool.tile([DK + 1, 2 * NQ], F32, tag="ce", bufs=2,
                                  name="ctxe")
                nc.vector.tensor_copy(ctxe[:], ctx_ps[0:DK + 1, :])
                dst = ctxT2[hp // 2]
                col = (hp % 2) * NQ
                for i, po in ((0, 0), (1, DK)):
                    rec = apool.tile([1, NQ], F32, tag="rec", bufs=2,
                                     name="rec")
                    nc.vector.reciprocal(
                        rec[:], ctxe[DK:DK + 1, i * NQ:(i + 1) * NQ])
                    bc = apool.tile([P, NQ], F32, tag="bc", bufs=2, name="bc")
                    nc.gpsimd.partition_broadcast(bc[:], rec[:])
                    nc.vector.scalar_tensor_tensor(
                        dst[po:po + DK, col:col + NQ],
                        ctxe[0:DK, i * NQ:(i + 1) * NQ],
                        CS, bc[0:DK, :], op0=ALU.mult, op1=ALU.mult)
                if STOP_AT == 2:
                    continue
                # stream Wo for the completed head-pair (fp8 DoubleRow)
                if hp % 2 == 1:
                    spx = hp // 2
                    src3 = ctxT2[spx].rearrange("p (i n) -> p i n", n=NQ)
                    for qt in range(4):
                        wops = psa.tile([P, D], F32, tag="ps", bufs=3,
                                        name="wo_ps")
                        nc.tensor.matmul(wops[:],
                                         src3[:, :, qt * P:(qt + 1) * P],
                                         wo_sb[:, 2 * spx:2 * spx + 2, :],
                                         start=True, stop=True, perf_mode=DR)
                        if hp == 1:
                            nc.vector.scalar_tensor_tensor(
                                h1acc[qt][:], wops[:], 1.0 / (WS * CS),
                                hq_sb[:, qt:qt + 1, :], op0=ALU.mult, op1=ALU.add)
                        else:
                            nc.vector.scalar_tensor_tensor(
                                h1acc[qt][:], wops[:], 1.0 / (WS * CS),
                                h1acc[qt][:], op0=ALU.mult, op1=ALU.add)

        if STOP_AT == 2:
            for qt in range(4):
                cv = h1pool.tile([P, D], F32, tag="dbg", bufs=2, name="cv")
                nc.vector.tensor_copy(
                    cv[:], ctxT2[qt // 2][:, (qt % 2) * NQ:(qt % 2 + 1) * NQ])
                nc.sync.dma_start(out[qt * P:(qt + 1) * P, :], cv[:])
            qkv_es.close()
            return
        if STOP_AT == 3:
            for qt in range(4):
                nc.sync.dma_start(out[qt * P:(qt + 1) * P, :], h1acc[qt][:])
            qkv_es.close()
            return

        # ---- LN1 (batched stats) + f32 transpose ----
        with tc.tile_pool(name="pspost", bufs=2, space="PSUM") as psw:
            s1 = h1pool.tile([P, 4], F32, tag="s1a", name="s1a")
            s2 = h1pool.tile([P, 4], F32, tag="s2a", name="s2a")
            for qt in range(4):
                nc.vector.reduce_sum(s1[:, qt:qt + 1], h1acc[qt][:],
                                     axis=mybir.AxisListType.X)
                xsq = h1pool.tile([P, D], F32, tag="xsq", bufs=4, name="xsq")
                eng = nc.vector if qt < 2 else nc.gpsimd
                eng.tensor_mul(xsq[:], h1acc[qt][:], h1acc[qt][:])
                nc.vector.reduce_sum(s2[:, qt:qt + 1], xsq[:],
                                     axis=mybir.AxisListType.X)
            rstd4, nmr4 = _stats4(nc, h1pool, s1, s2, eps_sb, "a")
            for qt in range(4):
                xn = h1pool.tile([P, D], F32, tag="xn", bufs=2, name="xn")
                nc.scalar.activation(xn[:], h1acc[qt][:], AF.Identity,
                                     bias=nmr4[:, qt:qt + 1],
                                     scale=rstd4[:, qt:qt + 1])
                eng = nc.vector if qt < 2 else nc.gpsimd
                eng.tensor_mul(h1_sb[qt][:], xn[:], g1l)
                eng.tensor_add(h1_sb[qt][:], h1_sb[qt][:], b1l)
                for i in range(4):
                    tp = psw.tile([P, P], F32, tag="tp", name="tp")
                    nc.tensor.transpose(tp[:], h1_sb[qt][:, i * P:(i + 1) * P],
                                        identf_sb[:])
                    nc.vector.tensor_copy(
                        h1T_sb[i][:, qt * P:(qt + 1) * P], tp[:])

        qkv_es.close()

        # ================= FFN (ft-interleaved) =================
        ffnp = es.enter_context(tc.tile_pool(name="ffnp", bufs=1))
        w1_sb = ffnp.tile([P, 4, DFF], BF, tag="w1")
        nc.sync.dma_start(w1_sb[:], w1P[:])
        w2_sb = ffnp.tile([P, 16, D], BF, tag="w2")
        nc.sync.dma_start(w2_sb[:], w2P[:])
        with tc.tile_pool(name="psffn", bufs=1, space="PSUM") as psf:
            ff_ps = [psf.tile([P, D], F32, tag=f"fa{i}", name=f"ff_ps{i}")
                     for i in range(4)]
            for ft in range(16):
                ps = psf.tile([P, NQ], F32, tag="pf", bufs=2, name="f_ps")
                for s in range(4):
                    nc.tensor.matmul(ps[:],
                                     w1_sb[:, s:s + 1, ft * P:(ft + 1) * P],
                                     h1T_sb[s][:], start=(s == 0),
                                     stop=(s == 3))
                fT = ffnp.tile([P, NQ], BF, tag="fT", bufs=3, name="fT")
                nc.scalar.activation(fT[:], ps[:], AF.Relu,
                                     bias=b1_sb[:, ft:ft + 1])
                for qt in range(4):
                    nc.tensor.matmul(ff_ps[qt][:],
                                     fT[:, qt * P:(qt + 1) * P],
                                     w2_sb[:, ft:ft + 1, :], start=(ft == 0),
                                     stop=False)
            # ---- +b2, then LN2 with batched stats ----
            s1 = h1pool.tile([P, 4], F32, tag="s1b", name="s1b")
            s2 = h1pool.tile([P, 4], F32, tag="s2b", name="s2b")
            x2 = []
            for qt in range(4):
                nc.tensor.matmul(ff_ps[qt][:], ones_sb[:], b2_sb[:],
                                 start=False, stop=True)
                x = h1pool.tile([P, D], F32, tag=f"x2{qt}", name=f"x2{qt}")
                nc.vector.scalar_tensor_tensor(x[:], ff_ps[qt][:], 0.0,
                                               h1_sb[qt][:], op0=ALU.add,
                                               op1=ALU.add,
                                               accum_out=s1[:, qt:qt + 1])
                xsq = h1pool.tile([P, D], F32, tag="xsq", bufs=4, name="xsq")
                eng = nc.vector if qt < 2 else nc.gpsimd
                eng.tensor_mul(xsq[:], x[:], x[:])
                nc.vector.reduce_sum(s2[:, qt:qt + 1], xsq[:],
                                     axis=mybir.AxisListType.X)
                x2.append(x)
            rstd4, nmr4 = _stats4(nc, h1pool, s1, s2, eps_sb, "b")
            for qt in range(4):
                xn = h1pool.tile([P, D], F32, tag="xn", bufs=2, name="xn")
                nc.scalar.activation(xn[:], x2[qt][:], AF.Identity,
                                     bias=nmr4[:, qt:qt + 1],
                                     scale=rstd4[:, qt:qt + 1])
                h2 = h1pool.tile([P, D], F32, tag="h2o", bufs=4, name="h2")
                eng = nc.vector if qt < 2 else nc.gpsimd
                eng.tensor_mul(h2[:], xn[:], g2l)
                eng.tensor_add(h2[:], h2[:], b2l)
                nc.sync.dma_start(out[qt * P:(qt + 1) * P, :], h2[:])


def _stats4(nc, pool, s1, s2, eps_sb, uid):
    """Batched LN stats: from per-qt sums s1,s2 [P,4] compute rstd4 and
    nmr4 = (-mean * rstd) [P,4]."""
    I32 = mybir.dt.int32
    nm = pool.tile([P, 4], F32, tag="nm4", bufs=2, name=f"nm4{uid}")
    nc.vector.tensor_scalar_mul(nm[:], s1[:], -1.0 / D)
    m2 = pool.tile([P, 4], F32, tag="m24", bufs=2, name=f"m24{uid}")
    nc.vector.tensor_mul(m2[:], nm[:], nm[:])
    var = pool.tile([P, 4], F32, tag="var4", bufs=2, name=f"var4{uid}")
    nc.vector.scalar_tensor_tensor(var[:], s2[:], 1.0 / D, m2[:],
                                   op0=ALU.mult, op1=ALU.subtract)
    ve = pool.tile([P, 4], F32, tag="ve4", bufs=2, name=f"ve4{uid}")
    nc.vector.tensor_scalar_add(ve[:], var[:], eps_sb[:])
    rstd = pool.tile([P, 4], F32, tag="rs4", bufs=2, name=f"rs4{uid}")
    nc.vector.tensor_single_scalar(rstd[:].bitcast(I32), ve[:].bitcast(I32),
                                   1, op=ALU.arith_shift_right)
    nc.vector.tensor_single_scalar(rstd[:].bitcast(I32), rstd[:].bitcast(I32),
                                   0x5F3759DF, op=ALU.subtract)
    nc.vector.tensor_single_scalar(rstd[:].bitcast(I32), rstd[:].bitcast(I32),
                                   -1, op=ALU.mult)
    tq = pool.tile([P, 4], F32, tag="tq4", bufs=2, name=f"tq4{uid}")
    for _ in range(3):
        nc.vector.tensor_mul(tq[:], rstd[:], rstd[:])
        nc.vector.tensor_mul(tq[:], tq[:], ve[:])
        nc.vector.tensor_scalar_mul(tq[:], tq[:], -0.5)
        nc.vector.tensor_scalar_add(tq[:], tq[:], 1.5)
        nc.vector.tensor_mul(rstd[:], rstd[:], tq[:])
    nmr = pool.tile([P, 4], F32, tag="nmr4", bufs=2, name=f"nmr4{uid}")
    nc.vector.tensor_mul(nmr[:], nm[:], rstd[:])
    return rstd, nmr


def _fold(xT):
    """[512, C] -> [128, 4, C] with d = slot*128 + p."""
    c = xT.shape[1]
    return np.ascontiguousarray(xT.reshape(4, P, c).transpose(1, 0, 2))


def _prep_inputs(inputs):
    f32 = np.float32
    h = np.asarray(inputs["h"], f32)
    adj = np.asarray(inputs["adj"])

    def bf(x):
        return np.ascontiguousarray(np.asarray(x, f32).astype(BF16))

    def f8(x):
        return np.ascontiguousarray(np.asarray(x, f32).astype(FP8))

    hT = np.ascontiguousarray(h.T)
    adjb = (adj != 0)
    np.fill_diagonal(adjb, True)
    adjb_bf = adjb.astype(BF16)

    wq, wk, wv, wo = (np.asarray(inputs[k], f32)
                      for k in ("Wq", "Wk", "Wv", "Wo"))
    w1, w2 = np.asarray(inputs["W1"], f32), np.asarray(inputs["W2"], f32)
    bv = np.asarray(inputs["bv"], f32)
    bo = np.asarray(inputs["bo"], f32)
    bo2 = bo + bv @ wo.T  # bv folded through Wo

    lnc = np.concatenate([
        np.broadcast_to(np.asarray(inputs[k], f32), (P, D))
        for k in ("ln1_g", "ln1_b", "ln2_g", "ln2_b")], axis=1)

    shared = {
        "hT2": f8(_fold(hT)),
        "wq2": f8(_fold(wq.T) * WS), "wk2": f8(_fold(wk.T) * WS),
        "wv2": f8(_fold(wv.T) * WS), "wo2": f8(_fold(wo.T) * WS),
        "w1P": bf(w1.T.reshape(4, P, DFF).transpose(1, 0, 2)),
        "w2P": bf(w2.T.reshape(16, P, D).transpose(1, 0, 2)),
        "bqP": np.ascontiguousarray(
            np.asarray(inputs["bq"], f32).reshape(4, P).T),
        "b1P": np.ascontiguousarray(
            np.asarray(inputs["b1"], f32).reshape(16, P).T),
        "b22": bf(np.asarray(inputs["b2"], f32)[None, :]),
        "lnc": np.ascontiguousarray(lnc),
        "identf": np.eye(P, dtype=f32),
    }
    in_maps = []
    for i in range(NCORES):
        r0 = i * NQ
        m = dict(shared)
        m["hqT2"] = f8(_fold(np.ascontiguousarray(hT[:, r0:r0 + NQ])))
        m["hqP"] = np.ascontiguousarray(
            (h[r0:r0 + NQ, :] + bo2).reshape(4, P, D).transpose(1, 0, 2))
        m["maskP"] = np.ascontiguousarray(
            adjb_bf[r0:r0 + NQ, :].T.reshape(32, P, NQ).transpose(1, 0, 2))
        in_maps.append(m)
    return in_maps


def kernel(**inputs) -> np.ndarray:
    global _CACHED, LAST_EXEC_NS
    if _CACHED is None:
        _CACHED = _build()
    nc = _CACHED
    in_maps = _prep_inputs(inputs)
    kw = {}
    if TRACE:
        kw = dict(trace=True, tmpdir=TRACE_DIR)
    res = run_bass_kernel_spmd(nc, in_maps, list(range(NCORES)), **kw)
    LAST_EXEC_NS = res.exec_time_ns
    return np.concatenate([res.results[i]["out"] for i in range(NCORES)],
                          axis=0)


# revision 19
# speedup vs baseline: 1.2209x; 1.0213x over previous
"""GraphTransformerLayer on 8 Trainium2 NeuronCores (Bass/Tile).

Sharding: 8-way along the query-node axis. Each core owns NQ=512 query rows,
computes full K/V projections (replicated), its slice of masked attention,
and its slice of the FFN. No collectives; the host concatenates the slices.

v3:
- Q/K/V and Wo matmuls in fp8e4 DoubleRow perf mode (2 k-subtiles per
  instruction, 0.5 cycles/row); weights host-folded to [128, 4, .], x32
  scaled (ctx x64) for fp8 range. bk dropped (exact under softmax), bv
  folded into host bo' = bo + bv @ Wo.T.
- Wo streams per head-pair into an SBUF f32 accumulator; ctx/wo PSUM tiles
  share one 4-deep ring so normalization lag never stalls the next pair.
- All DMA'd tensors host-packed to their exact SBUF layouts (contiguous,
  hardware-DGE friendly); issued in first-use order, w1/w2 before attention.
- LayerNorms use batched [128,4] stats + rstd across the 4 query tiles,
  ACT-engine center/scale, f32 PE transposes (no bf16 staging copy).
- FFN1/FFN2 interleaved per ft tile; per-qt LN2 + output DMA.
"""

import sys

if "/opt/trn_rl_repo" not in sys.path:
    sys.path.insert(0, "/opt/trn_rl_repo")

import numpy as np
import ml_dtypes

import concourse.bacc as bacc
import concourse.tile as tile
import concourse.mybir as mybir
from concourse.bass_utils import run_bass_kernel_spmd

BF16 = ml_dtypes.bfloat16
FP8 = ml_dtypes.float8_e4m3
F32 = mybir.dt.float32
BF = mybir.dt.bfloat16
F8 = mybir.dt.float8e4

N = 4096
D = 512
H = 8
DK = 64
DFF = 2048
NCORES = 8
NQ = N // NCORES
P = 128
EPS = 1e-5
WS = 32.0  # host weight pre-scale for fp8
CS = 64.0  # ctx pre-scale for fp8

ALU = mybir.AluOpType
AF = mybir.ActivationFunctionType
DR = mybir.MatmulPerfMode.DoubleRow

# set by test.py to capture a profile
TRACE = False
TRACE_DIR = None
LAST_EXEC_NS = None

# debug: truncate after a phase (1=proj, 2=attention ctx, 3=h1acc, 4=full)
STOP_AT = 4

_CACHED = None


def _build():
    nc = bacc.Bacc("TRN2", target_bir_lowering=False, debug=False,
                   num_devices=NCORES)

    # folded fp8 tensors: [128, 4, C]; d = slot*128 + p
    hT2 = nc.dram_tensor("hT2", [P, 4, N], F8, kind="ExternalInput").ap()
    hqT2 = nc.dram_tensor("hqT2", [P, 4, NQ], F8, kind="ExternalInput").ap()
    wq2 = nc.dram_tensor("wq2", [P, 4, D], F8, kind="ExternalInput").ap()
    wk2 = nc.dram_tensor("wk2", [P, 4, D], F8, kind="ExternalInput").ap()
    wv2 = nc.dram_tensor("wv2", [P, 4, D], F8, kind="ExternalInput").ap()
    wo2 = nc.dram_tensor("wo2", [P, 4, D], F8, kind="ExternalInput").ap()
    maskP = nc.dram_tensor("maskP", [P, 32, NQ], BF, kind="ExternalInput").ap()
    w1P = nc.dram_tensor("w1P", [P, 4, DFF], BF, kind="ExternalInput").ap()
    w2P = nc.dram_tensor("w2P", [P, 16, D], BF, kind="ExternalInput").ap()
    bqP = nc.dram_tensor("bqP", [P, 4], F32, kind="ExternalInput").ap()
    b1P = nc.dram_tensor("b1P", [P, 16], F32, kind="ExternalInput").ap()
    b22 = nc.dram_tensor("b22", [1, D], BF, kind="ExternalInput").ap()
    lnc = nc.dram_tensor("lnc", [P, 4 * D], F32, kind="ExternalInput").ap()
    hqP = nc.dram_tensor("hqP", [P, 4, D], F32, kind="ExternalInput").ap()
    identf = nc.dram_tensor("identf", [P, P], F32, kind="ExternalInput").ap()
    out = nc.dram_tensor("out", [NQ, D], F32, kind="ExternalOutput").ap()

    with tile.TileContext(nc) as tc:
        _emit(nc, tc, locals())
    nc.compile()
    return nc


def _emit(nc, tc, t):
    hT2, hqT2, maskP = t["hT2"], t["hqT2"], t["maskP"]
    wq2, wk2, wv2, wo2 = t["wq2"], t["wk2"], t["wv2"], t["wo2"]
    w1P, w2P = t["w1P"], t["w2P"]
    bqP, b1P, b22, lnc, hqP = t["bqP"], t["b1P"], t["b22"], t["lnc"], t["hqP"]
    identf, out = t["identf"], t["out"]

    from contextlib import ExitStack

    es = ExitStack()
    with es:
        cpool = es.enter_context(tc.tile_pool(name="const", bufs=1))
        h1pool = es.enter_context(tc.tile_pool(name="h1p", bufs=1))
        qkv_es = ExitStack()
        qkvpool = qkv_es.enter_context(tc.tile_pool(name="qkvp", bufs=1))
        mpool = qkv_es.enter_context(tc.tile_pool(name="maskp", bufs=1))
        proj_es = ExitStack()
        ppool = proj_es.enter_context(tc.tile_pool(name="projp", bufs=1))

        # ---- DMAs, first-use order, all contiguous host-packed ----
        wq_sb = ppool.tile([P, 4, D], F8, tag="wq")
        nc.sync.dma_start(wq_sb[:], wq2[:])
        hqT_sb = ppool.tile([P, 4, NQ], F8, tag="hqT")
        nc.sync.dma_start(hqT_sb[:], hqT2[:])
        bq_sb = cpool.tile([P, 4], F32, tag="bq")
        nc.sync.dma_start(bq_sb[:], bqP[:])
        wk_sb = ppool.tile([P, 4, D], F8, tag="wk")
        nc.sync.dma_start(wk_sb[:], wk2[:])
        hT_sb = ppool.tile([P, 4, N], F8, tag="hT")
        for ck in range(4):
            nc.sync.dma_start(hT_sb[:, :, ck * (N // 4):(ck + 1) * (N // 4)],
                              hT2[:, :, ck * (N // 4):(ck + 1) * (N // 4)])
        wv_sb = ppool.tile([P, 4, D], F8, tag="wv")
        nc.sync.dma_start(wv_sb[:], wv2[:])
        # mask in SBUF layout [128, mt, NQ], two halves on the gpsimd queue.
        # A dummy gpsimd read of hT_sb delays the 4MB mask transfer until the
        # critical-path hT/weight DMAs have drained (shared HBM bandwidth).
        mask_sb = mpool.tile([P, 32, NQ], BF, tag="mask")
        gate = cpool.tile([1, 4], F8, tag="gate")
        nc.gpsimd.tensor_copy(gate[:], hT_sb[0:1, :, 0:1])
        nc.gpsimd.dma_start(mask_sb[:, 0:16, :], maskP[:, 0:16, :])
        nc.gpsimd.dma_start(mask_sb[:, 16:32, :], maskP[:, 16:32, :])
        wo_sb = cpool.tile([P, 4, D], F8, tag="wo")
        nc.sync.dma_start(wo_sb[:], wo2[:])
        lnab = cpool.tile([P, 4 * D], F32, tag="lnc")
        nc.sync.dma_start(lnab[:], lnc[:])
        hq_sb = cpool.tile([P, 4, D], F32, tag="hq")
        nc.sync.dma_start(hq_sb[:], hqP[:])
        b1_sb = cpool.tile([P, 16], F32, tag="b1")
        nc.sync.dma_start(b1_sb[:], b1P[:])
        b2_sb = cpool.tile([1, D], BF, tag="b2")
        nc.sync.dma_start(b2_sb[:], b22[:])
        identf_sb = cpool.tile([P, P], F32, tag="idf")
        nc.sync.dma_start(identf_sb[:], identf[:])
        ones_sb = cpool.tile([1, P], BF, tag="ones")
        nc.vector.memset(ones_sb[:], 1.0)
        eps_sb = cpool.tile([P, 1], F32, tag="eps")
        nc.vector.memset(eps_sb[:], EPS)

        g1l = lnab[:, 0:D]
        b1l = lnab[:, D:2 * D]
        g2l = lnab[:, 2 * D:3 * D]
        b2l = lnab[:, 3 * D:4 * D]

        # ---- persistent attention state ----
        kT_sb = [qkvpool.tile([P, N], BF, tag=f"kt{i}", name=f"kT{i}")
                 for i in range(4)]
        qT_sb = [qkvpool.tile([P, NQ], BF, tag=f"qt{i}", name=f"qT{i}")
                 for i in range(4)]
        v_sb = [qkvpool.tile([P, H * (DK + 1)], BF, tag=f"v{i}", name=f"v{i}")
                for i in range(32)]
        for mt in range(32):
            vv = v_sb[mt].rearrange("p (h c) -> p h c", c=DK + 1)
            nc.vector.memset(vv[:, :, DK:DK + 1], 1.0)
        ctxT2 = [h1pool.tile([P, 2 * NQ], F8, tag=f"cx{i}", name=f"ctxT2{i}")
                 for i in range(2)]
        h1acc = [h1pool.tile([P, D], F32, tag=f"ha{i}", name=f"h1acc{i}")
                 for i in range(4)]
        h1_sb = [h1pool.tile([P, D], F32, tag=f"h1_{i}", name=f"h1_{i}")
                 for i in range(4)]
        h1T_sb = [h1pool.tile([P, NQ], BF, tag=f"h1T{i}", name=f"h1T{i}")
                  for i in range(4)]

        # ================= projections (fp8 DoubleRow) =================
        with tc.tile_pool(name="psproj", bufs=4, space="PSUM") as psp:
            for tt in range(4):
                ps = psp.tile([P, NQ], F32, tag="pp", name="ps_q")
                for sp in range(2):
                    nc.tensor.matmul(ps[:],
                                     wq_sb[:, 2 * sp:2 * sp + 2,
                                           tt * P:(tt + 1) * P],
                                     hqT_sb[:, 2 * sp:2 * sp + 2, :],
                                     start=(sp == 0), stop=(sp == 1),
                                     perf_mode=DR)
                nc.scalar.activation(qT_sb[tt][:], ps[:], AF.Identity,
                                     bias=bq_sb[:, tt:tt + 1], scale=1.0 / WS)
            for tt in range(4):
                for c in range(8):
                    ps = psp.tile([P, D], F32, tag="pp", name="ps_k")
                    for sp in range(2):
                        nc.tensor.matmul(ps[:],
                                         wk_sb[:, 2 * sp:2 * sp + 2,
                                               tt * P:(tt + 1) * P],
                                         hT_sb[:, 2 * sp:2 * sp + 2,
                                               c * D:(c + 1) * D],
                                         start=(sp == 0), stop=(sp == 1),
                                         perf_mode=DR)
                    nc.vector.tensor_scalar_mul(
                        kT_sb[tt][:, c * D:(c + 1) * D], ps[:], 1.0 / WS)
            for mc in range(32):
                ps = psp.tile([P, D], F32, tag="pp", name="ps_v")
                for sp in range(2):
                    nc.tensor.matmul(ps[:],
                                     hT_sb[:, 2 * sp:2 * sp + 2,
                                           mc * P:(mc + 1) * P],
                                     wv_sb[:, 2 * sp:2 * sp + 2, :],
                                     start=(sp == 0), stop=(sp == 1),
                                     perf_mode=DR)
                vv = v_sb[mc].rearrange("p (h c) -> p h c", c=DK + 1)
                nc.scalar.activation(vv[:, :, 0:DK],
                                     ps.rearrange("p (h c) -> p h c", c=DK),
                                     AF.Copy, scale=1.0 / WS)
        proj_es.close()

        if STOP_AT == 1:
            for qt in range(4):
                cv = h1pool.tile([P, D], F32, tag="dbg", bufs=2, name="cv")
                nc.vector.tensor_add(cv[:], kT_sb[qt][:, 0:D], qT_sb[qt][:])
                nc.vector.tensor_add(cv[:], cv[:], v_sb[qt * 8][:, 0:D])
                nc.sync.dma_start(out[qt * P:(qt + 1) * P, :], cv[:])
            qkv_es.close()
            return

        # ================= attention =================
        with tc.tile_pool(name="attp", bufs=1) as apool, \
             tc.tile_pool(name="psatt", bufs=1, space="PSUM") as psa:
            for hp in range(4):
                ctx_ps = psa.tile([P, 2 * NQ], F32, tag="pc", bufs=1,
                                  name="ctx_ps")
                for g in range(16):
                    sp = [psa.tile([P, 1024], F32, tag="ps", bufs=3,
                                   name="sc_ps") for _ in range(2)]
                    at = [apool.tile([P, 1024], BF, tag="at", bufs=6,
                                     name="at") for _ in range(2)]
                    for i, po in ((0, 0), (1, DK)):
                        for j in range(2):
                            mt = 2 * g + j
                            nc.tensor.matmul(
                                sp[i][:, j * NQ:(j + 1) * NQ],
                                kT_sb[hp][po:po + DK, mt * P:(mt + 1) * P],
                                qT_sb[hp][po:po + DK, :],
                                start=True, stop=True)
                    for i in range(2):
                        nc.scalar.activation(at[i][:], sp[i][:], AF.Exp,
                                             scale=0.125)
                        nc.vector.tensor_mul(
                            at[i][:], at[i][:],
                            mask_sb[:, 2 * g:2 * g + 2, :])
                    for i, h in ((0, 2 * hp), (1, 2 * hp + 1)):
                        for j in range(2):
                            mt = 2 * g + j
                            nc.tensor.matmul(
                                ctx_ps[0:DK + 1, i * NQ:(i + 1) * NQ],
                                v_sb[mt][:, h * 65:h * 65 + 65],
                                at[i][:, j * NQ:(j + 1) * NQ],
                                start=(mt == 0), stop=(mt == 31))
                # evict ctx+rowsums to SBUF fast (frees the psum bank for
                # the next head-pair), then normalize from SBUF
                ctxe = apool.tile([DK + 1, 2 * NQ], F32, tag="ce", bufs=2,
                                  name="ctxe")
                nc.vector.tensor_copy(ctxe[:], ctx_ps[0:DK + 1, :])
                dst = ctxT2[hp // 2]
                col = (hp % 2) * NQ
                for i, po in ((0, 0), (1, DK)):
                    rec = apool.tile([1, NQ], F32, tag="rec", bufs=2,
                                     name="rec")
                    nc.vector.reciprocal(
                        rec[:], ctxe[DK:DK + 1, i * NQ:(i + 1) * NQ])
                    bc = apool.tile([P, NQ], F32, tag="bc", bufs=2, name="bc")
                    nc.gpsimd.partition_broadcast(bc[:], rec[:])
                    nc.vector.scalar_tensor_tensor(
                        dst[po:po + DK, col:col + NQ],
                        ctxe[0:DK, i * NQ:(i + 1) * NQ],
                        CS, bc[0:DK, :], op0=ALU.mult, op1=ALU.mult)
                if STOP_AT == 2:
                    continue

        if STOP_AT == 2:
            for qt in range(4):
                cv = h1pool.tile([P, D], F32, tag="dbg", bufs=2, name="cv")
                nc.vector.tensor_copy(
                    cv[:], ctxT2[qt // 2][:, (qt % 2) * NQ:(qt % 2 + 1) * NQ])
                nc.sync.dma_start(out[qt * P:(qt + 1) * P, :], cv[:])
            qkv_es.close()
            return
        if STOP_AT == 3:
            for qt in range(4):
                nc.sync.dma_start(out[qt * P:(qt + 1) * P, :], h1acc[qt][:])
            qkv_es.close()
            return

        # ---- Wo + LN1 (batched stats) + f32 transpose ----
        src3 = [ctxT2[spx].rearrange("p (i n) -> p i n", n=NQ)
                for spx in range(2)]
        with tc.tile_pool(name="pspost", bufs=2, space="PSUM") as psw:
            s1 = h1pool.tile([P, 4], F32, tag="s1a", name="s1a")
            s2 = h1pool.tile([P, 4], F32, tag="s2a", name="s2a")
            for qt in range(4):
                wops = psw.tile([P, D], F32, tag="wo", bufs=2, name="wo_ps")
                for spx in range(2):
                    nc.tensor.matmul(wops[:],
                                     src3[spx][:, :, qt * P:(qt + 1) * P],
                                     wo_sb[:, 2 * spx:2 * spx + 2, :],
                                     start=(spx == 0), stop=(spx == 1),
                                     perf_mode=DR)
                nc.vector.scalar_tensor_tensor(
                    h1acc[qt][:], wops[:], 1.0 / (WS * CS),
                    hq_sb[:, qt:qt + 1, :], op0=ALU.mult, op1=ALU.add,
                    accum_out=s1[:, qt:qt + 1])
                xsq = h1pool.tile([P, D], F32, tag="xsq", bufs=4, name="xsq")
                eng = nc.vector if qt < 2 else nc.gpsimd
                eng.tensor_mul(xsq[:], h1acc[qt][:], h1acc[qt][:])
                nc.vector.reduce_sum(s2[:, qt:qt + 1], xsq[:],
                                     axis=mybir.AxisListType.X)
            rstd4, nmr4 = _stats4(nc, h1pool, s1, s2, eps_sb, "a")
            for qt in range(4):
                xn = h1pool.tile([P, D], F32, tag="xn", bufs=2, name="xn")
                nc.scalar.activation(xn[:], h1acc[qt][:], AF.Identity,
                                     bias=nmr4[:, qt:qt + 1],
                                     scale=rstd4[:, qt:qt + 1])
                eng = nc.vector if qt < 2 else nc.gpsimd
                eng.tensor_mul(h1_sb[qt][:], xn[:], g1l)
                eng.tensor_add(h1_sb[qt][:], h1_sb[qt][:], b1l)
                for i in range(4):
                    tp = psw.tile([P, P], F32, tag="tp", name="tp")
                    nc.tensor.transpose(tp[:], h1_sb[qt][:, i * P:(i + 1) * P],
                                        identf_sb[:])
                    nc.vector.tensor_copy(
                        h1T_sb[i][:, qt * P:(qt + 1) * P], tp[:])

        qkv_es.close()

        # ================= FFN (ft-interleaved) =================
        ffnp = es.enter_context(tc.tile_pool(name="ffnp", bufs=1))
        w1_sb = ffnp.tile([P, 4, DFF], BF, tag="w1")
        nc.sync.dma_start(w1_sb[:], w1P[:])
        w2_sb = ffnp.tile([P, 16, D], BF, tag="w2")
        nc.sync.dma_start(w2_sb[:], w2P[:])
        with tc.tile_pool(name="psffn", bufs=1, space="PSUM") as psf:
            ff_ps = [psf.tile([P, D], F32, tag=f"fa{i}", name=f"ff_ps{i}")
                     for i in range(4)]
            for ft in range(16):
                ps = psf.tile([P, NQ], F32, tag="pf", bufs=2, name="f_ps")
                for s in range(4):
                    nc.tensor.matmul(ps[:],
                                     w1_sb[:, s:s + 1, ft * P:(ft + 1) * P],
                                     h1T_sb[s][:], start=(s == 0),
                                     stop=(s == 3))
                fT = ffnp.tile([P, NQ], BF, tag="fT", bufs=3, name="fT")
                nc.scalar.activation(fT[:], ps[:], AF.Relu,
                                     bias=b1_sb[:, ft:ft + 1])
                for qt in range(4):
                    nc.tensor.matmul(ff_ps[qt][:],
                                     fT[:, qt * P:(qt + 1) * P],
                                     w2_sb[:, ft:ft + 1, :], start=(ft == 0),
                                     stop=False)
            # ---- +b2, then LN2 with batched stats ----
            s1 = h1pool.tile([P, 4], F32, tag="s1b", name="s1b")
            s2 = h1pool.tile([P, 4], F32, tag="s2b", name="s2b")
            x2 = []
            for qt in range(4):
                nc.tensor.matmul(ff_ps[qt][:], ones_sb[:], b2_sb[:],
                                 start=False, stop=True)
                x = h1pool.tile([P, D], F32, tag=f"x2{qt}", name=f"x2{qt}")
                nc.vector.scalar_tensor_tensor(x[:], ff_ps[qt][:], 0.0,
                                               h1_sb[qt][:], op0=ALU.add,
                                               op1=ALU.add,
                                               accum_out=s1[:, qt:qt + 1])
                xsq = h1pool.tile([P, D], F32, tag="xsq", bufs=4, name="xsq")
                eng = nc.vector if qt < 2 else nc.gpsimd
                eng.tensor_mul(xsq[:], x[:], x[:])
                nc.vector.reduce_sum(s2[:, qt:qt + 1], xsq[:],
                                     axis=mybir.AxisListType.X)
                x2.append(x)
            rstd4, nmr4 = _stats4(nc, h1pool, s1, s2, eps_sb, "b")
            for qt in range(4):
                xn = h1pool.tile([P, D], F32, tag="xn", bufs=2, name="xn")
                nc.scalar.activation(xn[:], x2[qt][:], AF.Identity,
                                     bias=nmr4[:, qt:qt + 1],
                                     scale=rstd4[:, qt:qt + 1])
                h2 = h1pool.tile([P, D], F32, tag="h2o", bufs=4, name="h2")
                eng = nc.vector if qt < 2 else nc.gpsimd
                eng.tensor_mul(h2[:], xn[:], g2l)
                eng.tensor_add(h2[:], h2[:], b2l)
                nc.sync.dma_start(out[qt * P:(qt + 1) * P, :], h2[:])


def _stats4(nc, pool, s1, s2, eps_sb, uid):
    """Batched LN stats: from per-qt sums s1,s2 [P,4] compute rstd4 and
    nmr4 = (-mean * rstd) [P,4]."""
    I32 = mybir.dt.int32
    nm = pool.tile([P, 4], F32, tag="nm4", bufs=2, name=f"nm4{uid}")
    nc.vector.tensor_scalar_mul(nm[:], s1[:], -1.0 / D)
    m2 = pool.tile([P, 4], F32, tag="m24", bufs=2, name=f"m24{uid}")
    nc.vector.tensor_mul(m2[:], nm[:], nm[:])
    var = pool.tile([P, 4], F32, tag="var4", bufs=2, name=f"var4{uid}")
    nc.vector.scalar_tensor_tensor(var[:], s2[:], 1.0 / D, m2[:],
                                   op0=ALU.mult, op1=ALU.subtract)
    ve = pool.tile([P, 4], F32, tag="ve4", bufs=2, name=f"ve4{uid}")
    nc.vector.tensor_scalar_add(ve[:], var[:], eps_sb[:])
    rstd = pool.tile([P, 4], F32, tag="rs4", bufs=2, name=f"rs4{uid}")
    nc.vector.tensor_single_scalar(rstd[:].bitcast(I32), ve[:].bitcast(I32),
                                   1, op=ALU.arith_shift_right)
    nc.vector.tensor_single_scalar(rstd[:].bitcast(I32), rstd[:].bitcast(I32),
                                   0x5F3759DF, op=ALU.subtract)
    nc.vector.tensor_single_scalar(rstd[:].bitcast(I32), rstd[:].bitcast(I32),
                                   -1, op=ALU.mult)
    tq = pool.tile([P, 4], F32, tag="tq4", bufs=2, name=f"tq4{uid}")
    for _ in range(3):
        nc.vector.tensor_mul(tq[:], rstd[:], rstd[:])
        nc.vector.tensor_mul(tq[:], tq[:], ve[:])
        nc.vector.tensor_scalar_mul(tq[:], tq[:], -0.5)
        nc.vector.tensor_scalar_add(tq[:], tq[:], 1.5)
        nc.vector.tensor_mul(rstd[:], rstd[:], tq[:])
    nmr = pool.tile([P, 4], F32, tag="nmr4", bufs=2, name=f"nmr4{uid}")
    nc.vector.tensor_mul(nmr[:], nm[:], rstd[:])
    return rstd, nmr


def _fold(xT):
    """[512, C] -> [128, 4, C] with d = slot*128 + p."""
    c = xT.shape[1]
    return np.ascontiguousarray(xT.reshape(4, P, c).transpose(1, 0, 2))


def _prep_inputs(inputs):
    f32 = np.float32
    h = np.asarray(inputs["h"], f32)
    adj = np.asarray(inputs["adj"])

    def bf(x):
        return np.ascontiguousarray(np.asarray(x, f32).astype(BF16))

    def f8(x):
        return np.ascontiguousarray(np.asarray(x, f32).astype(FP8))

    hT = np.ascontiguousarray(h.T)
    adjb = (adj != 0)
    np.fill_diagonal(adjb, True)
    adjb_bf = adjb.astype(BF16)

    wq, wk, wv, wo = (np.asarray(inputs[k], f32)
                      for k in ("Wq", "Wk", "Wv", "Wo"))
    w1, w2 = np.asarray(inputs["W1"], f32), np.asarray(inputs["W2"], f32)
    bv = np.asarray(inputs["bv"], f32)
    bo = np.asarray(inputs["bo"], f32)
    bo2 = bo + bv @ wo.T  # bv folded through Wo

    lnc = np.concatenate([
        np.broadcast_to(np.asarray(inputs[k], f32), (P, D))
        for k in ("ln1_g", "ln1_b", "ln2_g", "ln2_b")], axis=1)

    shared = {
        "hT2": f8(_fold(hT)),
        "wq2": f8(_fold(wq.T) * WS), "wk2": f8(_fold(wk.T) * WS),
        "wv2": f8(_fold(wv.T) * WS), "wo2": f8(_fold(wo.T) * WS),
        "w1P": bf(w1.T.reshape(4, P, DFF).transpose(1, 0, 2)),
        "w2P": bf(w2.T.reshape(16, P, D).transpose(1, 0, 2)),
        "bqP": np.ascontiguousarray(
            np.asarray(inputs["bq"], f32).reshape(4, P).T),
        "b1P": np.ascontiguousarray(
            np.asarray(inputs["b1"], f32).reshape(16, P).T),
        "b22": bf(np.asarray(inputs["b2"], f32)[None, :]),
        "lnc": np.ascontiguousarray(lnc),
        "identf": np.eye(P, dtype=f32),
    }
    in_maps = []
    for i in range(NCORES):
        r0 = i * NQ
        m = dict(shared)
        m["hqT2"] = f8(_fold(np.ascontiguousarray(hT[:, r0:r0 + NQ])))
        m["hqP"] = np.ascontiguousarray(
            (h[r0:r0 + NQ, :] + bo2).reshape(4, P, D).transpose(1, 0, 2))
        m["maskP"] = np.ascontiguousarray(
            adjb_bf[r0:r0 + NQ, :].T.reshape(32, P, NQ).transpose(1, 0, 2))
        in_maps.append(m)
    return in_maps


def kernel(**inputs) -> np.ndarray:
    global _CACHED, LAST_EXEC_NS
    if _CACHED is None:
        _CACHED = _build()
    nc = _CACHED
    in_maps = _prep_inputs(inputs)
    kw = {}
    if TRACE:
        kw = dict(trace=True, tmpdir=TRACE_DIR)
    res = run_bass_kernel_spmd(nc, in_maps, list(range(NCORES)), **kw)
    LAST_EXEC_NS = res.exec_time_ns
    return np.concatenate([res.results[i]["out"] for i in range(NCORES)],
                          axis=0)
